# revision 1
# baseline (speedup 1.0000x reference)
"""Multi-head attention (RoPE, non-causal) on 8 Trainium2 NeuronCores.

Problem: x[4,2048,2048] fp32; wq/wk/wv/wo [2048,2048]; biases [2048].
  q,k,v = x@w.T+b per 16 heads of dim 128; rope(q,k); softmax(q k^T/sqrt(128));
  out = (attn@v)@wo.T + bo.

Sharding: core c = 2*b + g -> batch b, head-group g (8 heads each).
Each core computes a partial output (its 8 heads) for its batch over the full
sequence; the host sums the partials (the wo contraction splits cleanly over
head groups) and adds bo_eff = bo + wo@bv (the V bias folds out exactly
because softmax rows sum to 1).

v2 (software-pipelined): a single slot loop drives the attention inner loop
(scores matmul -> exp -> ctx matmul -> pden add) for head h while "filler"
pieces -- the Q/K projection + rope of head h+1, the softmax-denominator
chain of head h-1, and output-projection (P3) groups for already-finished
heads -- are interleaved between the attention matmuls in engine issue
order.  This keeps the PE dense (no >3.4us gaps, so the HAM clock stays
warm) and hides the ACT exp latency under PE work.  The P3 contraction over
the core's 8 heads is split into three partial outputs (heads 0-3 / 4-5 /
6-7) so most of it runs as filler during heads 4-7; the host sums the
partials.  The softmax reciprocal runs on a [128,16] transposed layout
(0.2us) instead of the broadcast [128,2048] one (12.9us).
"""

import sys

if "/opt/trn_rl_repo" not in sys.path:
    sys.path.insert(0, "/opt/trn_rl_repo")

import ml_dtypes
import numpy as np

import concourse.bass as bass
import concourse.tile as tile
from concourse import bacc, mybir
from concourse.bass_utils import run_bass_kernel_spmd

F32 = mybir.dt.float32
BF16 = mybir.dt.bfloat16
NPBF = ml_dtypes.bfloat16

B, S, D = 4, 2048, 2048
H = 16
DH = 128
HL = 8  # heads per core
KO = D // 128  # 16 k-chunks
TB = S // 128  # 16 t-chunks
ROPE_THETA = 10000.0
QSCALE = 1.0 / np.sqrt(DH)

# P3 partial-output head ranges and when each is emitted:
#   partial 0: heads 0-3, filler during heads 4-6
#   partial 1: heads 4-5, filler during head 7
#   partial 2: head 6 and partial 3: head 7, in the tail (partial 2's
#   matmuls cover the latency of head 7's denominator chain)
P3_PARTS = [(0, 4), (4, 6), (6, 7), (7, 8)]

_NC_CACHE = {}


def build_nc():
    nc = bacc.Bacc()

    xt_d = nc.declare_dram_parameter("xt", [KO, 128, S], BF16, isOutput=False)
    wq_d = nc.declare_dram_parameter("wq", [HL, KO, 128, 128], BF16, isOutput=False)
    wk_d = nc.declare_dram_parameter("wk", [HL, KO, 128, 128], BF16, isOutput=False)
    wv_d = nc.declare_dram_parameter("wv", [KO, 128, HL * DH], BF16, isOutput=False)
    wo_d = nc.declare_dram_parameter("wo", [HL, 128, D], BF16, isOutput=False)
    cos_d = nc.declare_dram_parameter("cosT", [128, S], BF16, isOutput=False)
    sin_d = nc.declare_dram_parameter("sinS", [128, S], BF16, isOutput=False)
    bq_d = nc.declare_dram_parameter("bq", [128, HL], F32, isOutput=False)
    bk_d = nc.declare_dram_parameter("bk", [128, HL], F32, isOutput=False)
    out_ds = [
        nc.declare_dram_parameter(f"out{p}", [S, D], BF16, isOutput=True)
        for p in range(len(P3_PARTS))
    ]

    v_d = nc.dram_tensor("v_spill", [TB, 128, HL * DH], BF16)
    ct_d = nc.dram_tensor("ct_spill", [HL, 128, S], BF16)
    den_d = nc.dram_tensor("den_bounce", [HL, 1, S], F32)
    rden_d = nc.dram_tensor("rden_bounce", [HL, 1, S], BF16)

    with tile.TileContext(nc) as tc:
        with (
            tc.tile_pool(name="xt_pool", bufs=1) as xt_pool,
            tc.tile_pool(name="const_pool", bufs=1) as const_pool,
            tc.tile_pool(name="w_pool", bufs=4) as w_pool,
            tc.tile_pool(name="qs_pool", bufs=2) as qs_pool,
            tc.tile_pool(name="rot_pool", bufs=2) as rot_pool,
            tc.tile_pool(name="qb_pool", bufs=4) as qb_pool,
            tc.tile_pool(name="v2_pool", bufs=2) as v2_pool,
            tc.tile_pool(name="et_pool", bufs=2) as et_pool,
            tc.tile_pool(name="pden_pool", bufs=2) as pden_pool,
            tc.tile_pool(name="den_pool", bufs=2) as den_pool,
            tc.tile_pool(name="norm_pool", bufs=2) as norm_pool,
            tc.tile_pool(name="psum", bufs=1, space="PSUM") as psum,
        ):
            # ---------------- prologue DMAs ----------------
            # (xt quarters and wv halves interleaved inside the V-phase
            #  block below so the first V matmuls start after ~4MB of DMA)
            xt_sb = xt_pool.tile([128, KO, S], BF16)

            cos_sb = const_pool.tile([128, S], BF16)
            sin_sb = const_pool.tile([128, S], BF16)
            bq_sb = const_pool.tile([128, HL], F32)
            bk_sb = const_pool.tile([128, HL], F32)
            ones_sb = const_pool.tile([128, 1], BF16)
            nc.sync.dma_start(out=cos_sb[:], in_=cos_d[:])
            nc.sync.dma_start(out=sin_sb[:], in_=sin_d[:])
            nc.sync.dma_start(out=bq_sb[:], in_=bq_d[:])
            nc.sync.dma_start(out=bk_sb[:], in_=bk_d[:])
            nc.vector.memset(ones_sb[:], 1.0)

            # per-head weight tiles (ring of 4: two heads in flight)
            w_tiles = {}

            def dma_w(h):
                for nm, w_d in (("q", wq_d), ("k", wk_d)):
                    t = w_pool.tile([128, KO, 128], BF16, tag="w", bufs=4, name=f"w_{nm}")
                    nc.sync.dma_start(
                        out=t[:], in_=w_d[h].rearrange("k p m -> p k m")
                    )
                    w_tiles[(h, nm)] = t

            dma_w(0)
            dma_w(1)

            v2_tiles = {}

            def dma_v2(h):
                t = v2_pool.tile([128, TB, DH], BF16, name="v2")
                nc.sync.dma_start(
                    out=t[:],
                    in_=v_d[:, :, h * DH : (h + 1) * DH].rearrange("t p m -> p t m"),
                )
                v2_tiles[h] = t

            # qb tiles (rope'd q^T / k^T, bf16, ring of 4)
            qb_tiles = {}
            pden_tiles = {}

            # ---------- filler piece machinery ----------
            def proj_pieces(h):
                """Q/K projection + rope for head h, as a list of closures."""
                pieces = []
                state = {}

                def chunk_first(nm, n, h=h):
                    def run():
                        w_sb = w_tiles[(h, nm)]
                        ps = psum.tile([128, 512], F32, tag="proj", bufs=2, name="proj_ps")
                        state[(nm, n)] = ps
                        for ko in range(8):
                            nc.tensor.matmul(
                                ps[:],
                                w_sb[:, ko, :],
                                xt_sb[:, ko, n * 512 : (n + 1) * 512],
                                start=(ko == 0),
                                stop=False,
                            )

                    return run

                def chunk_second(nm, n, h=h):
                    def run():
                        w_sb = w_tiles[(h, nm)]
                        ps = state[(nm, n)]
                        for ko in range(8, KO):
                            nc.tensor.matmul(
                                ps[:],
                                w_sb[:, ko, :],
                                xt_sb[:, ko, n * 512 : (n + 1) * 512],
                                start=False,
                                stop=(ko == KO - 1),
                            )
                        b_sb, scale = (
                            (bq_sb, QSCALE) if nm == "q" else (bk_sb, 1.0)
                        )
                        qs = state[("qs", nm)]
                        nc.vector.tensor_scalar(
                            out=qs[:, n * 512 : (n + 1) * 512],
                            in0=ps[:],
                            scalar1=scale,
                            scalar2=b_sb[:, h : h + 1],
                            op0=mybir.AluOpType.mult,
                            op1=mybir.AluOpType.add,
                        )

                    return run

                def rot_dma(nm):
                    def run():
                        qs = state[("qs", nm)]
                        rot = rot_pool.tile([128, S], BF16, tag="rot", bufs=2, name=f"rot_{nm}")
                        state[("rot", nm)] = rot
                        nc.sync.dma_start(out=rot[0:64, :], in_=qs[64:128, :])
                        nc.sync.dma_start(out=rot[64:128, :], in_=qs[0:64, :])

                    return run

                def rope(nm, h=h):
                    def run():
                        qs = state[("qs", nm)]
                        rot = state[("rot", nm)]
                        nc.vector.tensor_mul(out=qs[:], in0=qs[:], in1=cos_sb[:])
                        nc.vector.tensor_mul(out=rot[:], in0=rot[:], in1=sin_sb[:])
                        qb = qb_pool.tile([128, S], BF16, tag="qb", bufs=4, name=f"qb_{nm}")
                        nc.vector.tensor_add(out=qb[:], in0=qs[:], in1=rot[:])
                        qb_tiles[(h, nm)] = qb

                    return run

                for nm in ("q", "k"):

                    def alloc_qs(nm=nm):
                        state[("qs", nm)] = qs_pool.tile([128, S], BF16, tag="qs", bufs=2, name=f"qs_{nm}")

                    pieces.append(alloc_qs)
                    for n in range(4):
                        pieces.append(chunk_first(nm, n))
                        pieces.append(chunk_second(nm, n))
                    pieces.append(rot_dma(nm))
                    pieces.append(rope(nm))
                return pieces

            def den_pieces(h):
                """Softmax denominator + ct normalize/spill for head h."""
                pieces = []
                state = {}

                def den_mm():
                    pden = pden_tiles[h]
                    for n in range(4):
                        dps = psum.tile([1, 512], F32, tag="proj", bufs=2, name="dps")
                        nc.tensor.matmul(
                            dps[:],
                            ones_sb[:],
                            pden[:, n * 512 : (n + 1) * 512],
                            start=True,
                            stop=True,
                        )
                        dsb = den_pool.tile([1, 512], F32, tag="den", bufs=1, name="dsb")
                        nc.scalar.copy(out=dsb[:], in_=dps[:])
                        nc.sync.dma_start(
                            out=den_d[h][:, n * 512 : (n + 1) * 512], in_=dsb[:]
                        )

                def den_out():
                    # transposed read-back: [128, 16] so reciprocal is cheap
                    d128 = den_pool.tile([128, 16], F32, tag="d128", bufs=2, name="d128")
                    state["d128"] = d128
                    den_ap = den_d[h]
                    src = bass.AP(
                        tensor=den_ap.tensor,
                        offset=den_ap.offset,
                        ap=[[16, 128], [1, 16]],
                    )
                    nc.sync.dma_start(out=d128[:], in_=src)

                def recip():
                    d128 = state["d128"]
                    nc.vector.reciprocal(out=d128[:], in_=d128[:])
                    d128b = den_pool.tile([128, 16], BF16, tag="d128b", bufs=2, name="d128b")
                    nc.vector.tensor_copy(out=d128b[:], in_=d128[:])
                    rden_ap = rden_d[h]
                    dst = bass.AP(
                        tensor=rden_ap.tensor,
                        offset=rden_ap.offset,
                        ap=[[16, 128], [1, 16]],
                    )
                    nc.sync.dma_start(out=dst, in_=d128b[:])
                    # broadcast back across partitions
                    bc = norm_pool.tile([128, S], BF16, tag="bc", bufs=1, name="bc")
                    state["bc"] = bc
                    bsrc = bass.AP(
                        tensor=rden_ap.tensor,
                        offset=rden_ap.offset,
                        ap=[[0, 128]] + list(rden_ap.ap[1:]),
                    )
                    nc.sync.dma_start(out=bc[:], in_=bsrc)

                def norm():
                    cu = state["cu"]
                    bc = state["bc"]
                    ct_sb = norm_pool.tile([128, S], BF16, tag="ct", bufs=1, name="ct_sb")
                    nc.vector.tensor_mul(out=ct_sb[:], in0=cu[:], in1=bc[:])
                    nc.sync.dma_start(out=ct_d[h], in_=ct_sb[:])

                pieces.append(den_mm)
                pieces.append(den_out)
                pieces.append(recip)
                pieces.append(norm)
                return pieces, state

            def p3_pieces(part):
                """Output-projection groups for P3 partial `part`."""
                c0, c1 = P3_PARTS[part]
                ncs = c1 - c0
                pieces = []
                state = {}

                def cts_dma(m):
                    def run():
                        t = cts_pool.tile([128, ncs, 128], BF16, name="cts")
                        state[("cts", m)] = t
                        nc.sync.dma_start(
                            out=t[:],
                            in_=ct_d[c0:c1, :, m * 128 : (m + 1) * 128].rearrange(
                                "c p m2 -> p c m2"
                            ),
                        )

                    return run

                def alloc_osb(m):
                    def run():
                        state[("osb", m)] = osb_pool.tile([128, D], BF16, name="osb")

                    return run

                def group(m, n):
                    def run():
                        cts = state[("cts", m)]
                        ps = psum.tile([128, 512], F32, tag="proj", bufs=2, name="p3_ps")
                        for i in range(ncs):
                            nc.tensor.matmul(
                                ps[:],
                                cts[:, i, :],
                                wo_sb[:, c0 + i, n * 512 : (n + 1) * 512],
                                start=(i == 0),
                                stop=(i == ncs - 1),
                            )
                        osb = state[("osb", m)]
                        nc.vector.tensor_copy(
                            out=osb[:, n * 512 : (n + 1) * 512], in_=ps[:]
                        )
                        if n == 3:
                            nc.sync.dma_start(
                                out=out_ds[part][m * 128 : (m + 1) * 128, :],
                                in_=osb[:],
                            )

                    return run

                pieces.append(cts_dma(0))
                for m in range(TB):
                    pieces.append(alloc_osb(m))
                    if m + 1 < TB:
                        pieces.append(cts_dma(m + 1))
                    for n in range(4):
                        pieces.append(group(m, n))
                return pieces

            # ---------------- V phase (with head-0 projection as filler) ---
            with tc.tile_pool(name="wv_pool", bufs=1) as wv_pool, tc.tile_pool(
                name="vout_pool", bufs=3
            ) as vout_pool:
                wv_sb = wv_pool.tile([128, KO, HL * DH], BF16)

                def xt_dma(q):
                    sl = slice(q * 512, (q + 1) * 512)
                    nc.sync.dma_start(
                        out=xt_sb[:, :, sl],
                        in_=xt_d[:, :, sl].rearrange("k p s -> p k s"),
                    )

                def wv_dma(half):
                    sl = slice(half * 512, (half + 1) * 512)
                    nc.sync.dma_start(
                        out=wv_sb[:, :, sl],
                        in_=wv_d[:, :, sl].rearrange("k p m -> p k m"),
                    )

                xt_dma(0)
                wv_dma(0)
                xt_dma(1)
                wv_dma(1)
                xt_dma(2)
                xt_dma(3)

                vfill = proj_pieces(0)
                vi = 0
                for nf in range(2):
                    for tb in range(TB):
                        vps = psum.tile([128, 512], F32, tag="sc", bufs=2)
                        for ko in range(KO):
                            nc.tensor.matmul(
                                vps[:],
                                xt_sb[:, ko, tb * 128 : (tb + 1) * 128],
                                wv_sb[:, ko, nf * 512 : (nf + 1) * 512],
                                start=(ko == 0),
                                stop=(ko == KO - 1),
                            )
                        vsb = vout_pool.tile([128, 512], BF16)
                        nc.vector.tensor_copy(out=vsb[:], in_=vps[:])
                        nc.sync.dma_start(
                            out=v_d[tb, :, nf * 512 : (nf + 1) * 512], in_=vsb[:]
                        )
                        # ~20 filler pieces over 32 V units
                        unit = nf * TB + tb
                        want = ((unit + 1) * len(vfill)) // 32
                        while vi < want:
                            vfill[vi]()
                            vi += 1
                        if nf == 0 and tb == TB - 1:
                            dma_v2(0)
                while vi < len(vfill):
                    vfill[vi]()
                    vi += 1

            # ---------------- head loop ----------------
            with (
                tc.tile_pool(name="wo_pool", bufs=1) as wo_pool,
                tc.tile_pool(name="cts_pool", bufs=2) as cts_pool,
                tc.tile_pool(name="osb_pool", bufs=2) as osb_pool,
            ):
              wo_sb = wo_pool.tile([128, HL, D], BF16)
              for n in range(4):
                sl = slice(n * 512, (n + 1) * 512)
                nc.sync.dma_start(
                    out=wo_sb[:, :, sl],
                    in_=wo_d[:, :, sl].rearrange("c p m -> p c m"),
                )
              den_states = {}
              p3_p0 = p3_pieces(0)
              p0_cuts = [0, len(p3_p0) // 3, 2 * len(p3_p0) // 3, len(p3_p0)]
              for h in range(HL):
                  # filler: early pieces (weight DMAs, prev head's softmax
                  # denominator, next head's projection+rope) must finish by
                  # slot 12 so the next head's attention starts stall-free;
                  # late pieces (P3 groups) spread over all 16 slots.
                  fill = []
                  fill_late = []
                  if h >= 1:
                      dp, _st = den_states[h - 1]
                      fill.extend(dp)
                  if h + 2 < HL:
                      fill.append(lambda hh=h + 2: dma_w(hh))
                  if h + 1 < HL:
                      fill.append(lambda hh=h + 1: dma_v2(hh))
                      fill.extend(proj_pieces(h + 1))
                  if h in (4, 5, 6):
                      # spread partial-0 groups over heads 4-6
                      fill_late.extend(p3_p0[p0_cuts[h - 4] : p0_cuts[h - 3]])
                  if h == 7:
                      fill_late.extend(p3_pieces(1))

                  qt = qb_tiles[(h, "q")]
                  kt = qb_tiles[(h, "k")]
                  v2_sb = v2_tiles[h]

                  ctx_ps = psum.tile([128, S], F32, tag="ctx", bufs=1)
                  pden = pden_pool.tile([128, S], BF16)
                  pden_tiles[h] = pden

                  fi = 0
                  li = 0
                  for tb in range(TB):
                      et = et_pool.tile([128, S], BF16)
                      for n in range(4):
                          sc = psum.tile([128, 512], F32, tag="sc", bufs=2)
                          nc.tensor.matmul(
                              sc[:],
                              kt[:, tb * 128 : (tb + 1) * 128],
                              qt[:, n * 512 : (n + 1) * 512],
                              start=True,
                              stop=True,
                          )
                          nc.scalar.activation(
                              out=et[:, n * 512 : (n + 1) * 512],
                              in_=sc[:],
                              func=mybir.ActivationFunctionType.Exp,
                          )
                      # filler between scores and ctx
                      want = min(len(fill), ((tb + 1) * len(fill) + 11) // 12)
                      while fi < want:
                          fill[fi]()
                          fi += 1
                      # fill_late starts at slot 4: the first P3 group of a
                      # partial reads ct_spill written by den pieces that are
                      # emitted in this head's slots 0-2 (program-order
                      # read-after-write requirement)
                      if tb >= 4:
                          want_l = ((tb - 3) * len(fill_late)) // (TB - 4)
                          while li < want_l:
                              fill_late[li]()
                              li += 1
                      for n in range(4):
                          nc.tensor.matmul(
                              ctx_ps[:, n * 512 : (n + 1) * 512],
                              v2_sb[:, tb, :],
                              et[:, n * 512 : (n + 1) * 512],
                              start=(tb == 0),
                              stop=(tb == TB - 1),
                          )
                      if tb == 0:
                          nc.vector.tensor_copy(out=pden[:], in_=et[:])
                      else:
                          nc.vector.tensor_add(out=pden[:], in0=pden[:], in1=et[:])

                  # free ctx PSUM quickly: unnormalized ctx^T to SBUF (bf16)
                  dp, st = den_pieces(h)
                  den_states[h] = (dp, st)
                  cu = norm_pool.tile([128, S], BF16, tag="cu", bufs=1)
                  nc.scalar.copy(out=cu[:], in_=ctx_ps[:])
                  st["cu"] = cu

              # ---------------- tail: den(7) + P3 partial 2 ----------------
              for piece in den_states[7][0]:
                  piece()
              for piece in p3_pieces(2):
                  piece()
              for piece in p3_pieces(3):
                  piece()

    nc.finalize()
    return nc


def _get_nc():
    if "nc" not in _NC_CACHE:
        _NC_CACHE["nc"] = build_nc()
    return _NC_CACHE["nc"]


def _rope_tables():
    inv_freq = 1.0 / (ROPE_THETA ** (np.arange(0, DH, 2, dtype=np.float32) / DH))
    freqs = np.arange(S, dtype=np.float32)[:, None] * inv_freq[None, :]
    emb = np.concatenate([freqs, freqs], axis=-1)  # [S, 128]
    cosT = np.ascontiguousarray(np.cos(emb).T.astype(NPBF))  # [128, S]
    sinS = np.sin(emb).T.astype(np.float32).copy()
    sinS[0:64, :] *= -1.0  # sign-folded rotate_half
    return cosT, np.ascontiguousarray(sinS.astype(NPBF))


def kernel(x, wq, bq, wk, bk, wv, bv, wo, bo, _trace=False, _tmpdir=None):
    x = np.asarray(x, dtype=np.float32)
    wq = np.asarray(wq, dtype=np.float32)
    wk = np.asarray(wk, dtype=np.float32)
    wv = np.asarray(wv, dtype=np.float32)
    wo = np.asarray(wo, dtype=np.float32)
    bq = np.asarray(bq, dtype=np.float32)
    bk = np.asarray(bk, dtype=np.float32)
    bv = np.asarray(bv, dtype=np.float32)
    bo = np.asarray(bo, dtype=np.float32)

    nc = _get_nc()
    cosT, sinS = _rope_tables()

    def qk_pack(w, g):
        ws = w[g * 1024 : (g + 1) * 1024, :]
        return np.ascontiguousarray(
            ws.reshape(HL, 128, KO, 128).transpose(0, 2, 3, 1).astype(NPBF)
        )

    packs = []
    for g in range(2):
        wv_s = wv[g * 1024 : (g + 1) * 1024, :]
        wv_p = np.ascontiguousarray(
            wv_s.reshape(HL * DH, KO, 128).transpose(1, 2, 0).astype(NPBF)
        )
        wo_s = wo[:, g * 1024 : (g + 1) * 1024]
        wo_p = np.ascontiguousarray(
            wo_s.reshape(D, HL, 128).transpose(1, 2, 0).astype(NPBF)
        )
        bq_p = np.ascontiguousarray(
            (bq[g * 1024 : (g + 1) * 1024] * QSCALE).reshape(HL, 128).T
        )
        bk_p = np.ascontiguousarray(bk[g * 1024 : (g + 1) * 1024].reshape(HL, 128).T)
        packs.append(
            dict(
                wq=qk_pack(wq, g),
                wk=qk_pack(wk, g),
                wv=wv_p,
                wo=wo_p,
                bq=bq_p,
                bk=bk_p,
            )
        )

    in_maps = []
    xts = [
        np.ascontiguousarray(x[b].T.astype(NPBF)).reshape(KO, 128, S)
        for b in range(B)
    ]
    for c in range(8):
        b, g = c // 2, c % 2
        m = dict(packs[g])
        m["xt"] = xts[b]
        m["cosT"] = cosT
        m["sinS"] = sinS
        in_maps.append(m)

    res = run_bass_kernel_spmd(
        nc,
        in_maps,
        core_ids=list(range(8)),
        trace=_trace,
        tmpdir=_tmpdir,
    )

    bo_eff = bo + wo @ bv
    out = np.empty((B, S, D), dtype=np.float32)
    for b in range(B):
        acc = np.zeros((S, D), dtype=np.float32)
        for c in (2 * b, 2 * b + 1):
            for p in range(len(P3_PARTS)):
                acc += res.results[c][f"out{p}"].astype(np.float32)
        out[b] = acc + bo_eff[None, :]
    if _trace:
        kernel.last_result = res
    return out



# revision 10
# speedup vs baseline: 1.0357x; 1.0357x over previous
"""Multi-head attention (RoPE, non-causal) on 8 Trainium2 NeuronCores.

Problem: x[4,2048,2048] fp32; wq/wk/wv/wo [2048,2048]; biases [2048].
  q,k,v = x@w.T+b per 16 heads of dim 128; rope(q,k); softmax(q k^T/sqrt(128));
  out = (attn@v)@wo.T + bo.

Sharding: core c = 2*b + g -> batch b, head-group g (8 heads each).
Each core computes a partial output (its 8 heads) for its batch over the full
sequence; the host sums the partials (the wo contraction splits cleanly over
head groups) and adds bo_eff = bo + wo@bv (the V bias folds out exactly
because softmax rows sum to 1).

v3 (tail + engine-balance rework over v2's software pipeline):
  * P3 partials regrouped to [(0,4),(4,7),(7,8)]: partial 0 (heads 0-3,
    DRAM-bounced ct) fills heads 4-6; partial 1 (heads 4-6, normalized ct
    kept in SBUF - no spill/reload) fills head 7; partial 2 (head 7) runs
    in the tail, its matmuls emitted after the leftover partial-1 pieces so
    the PE stays busy under head 7's softmax-denominator DMA chain.
  * P3 PSUM->SBUF copies alternate between the Vector and Scalar engines
    (v2 put all of them on Vector, which serialized the tail at CAST speed).
  * Startup: wv streams on the Scalar DMA queue in parallel with xt on the
    Sync queue, and a burst of dummy matmuls warms the PE HAM clock while
    the first 4MB lands, so the first real matmul runs at 2.4GHz ~14us in.
  * The unnormalized-ctx PSUM->SBUF copy is split in 4 so ctx PSUM banks
    free progressively for the next head's first ctx matmuls.
"""

import sys

if "/opt/trn_rl_repo" not in sys.path:
    sys.path.insert(0, "/opt/trn_rl_repo")

import ml_dtypes
import numpy as np

import concourse.bass as bass
import concourse.tile as tile
from concourse import bacc, mybir
from concourse.bass_utils import run_bass_kernel_spmd

F32 = mybir.dt.float32
BF16 = mybir.dt.bfloat16
NPBF = ml_dtypes.bfloat16

B, S, D = 4, 2048, 2048
H = 16
DH = 128
HL = 8  # heads per core
KO = D // 128  # 16 k-chunks
TB = S // 128  # 16 t-chunks
ROPE_THETA = 10000.0
QSCALE = 1.0 / np.sqrt(DH)

# P3 partial-output head ranges:
#   partial 0: heads 0-3 (ct bounced via DRAM), filler during heads 4-6
#   partial 1: heads 4-6 (ct in SBUF), filler during head 7 + tail overlap
#   partial 2: head 7 (ct in SBUF), tail
P3_PARTS = [(0, 4), (4, 7), (7, 8)]

_NC_CACHE = {}


def build_nc():
    nc = bacc.Bacc()

    xt_d = nc.declare_dram_parameter("xt", [KO, 128, S], BF16, isOutput=False)
    wq_d = nc.declare_dram_parameter("wq", [HL, KO, 128, 128], BF16, isOutput=False)
    wk_d = nc.declare_dram_parameter("wk", [HL, KO, 128, 128], BF16, isOutput=False)
    wv_d = nc.declare_dram_parameter("wv", [KO, 128, HL * DH], BF16, isOutput=False)
    wo_d = nc.declare_dram_parameter("wo", [HL, 128, D], BF16, isOutput=False)
    cos_d = nc.declare_dram_parameter("cosT", [128, S], BF16, isOutput=False)
    sin_d = nc.declare_dram_parameter("sinS", [128, S], BF16, isOutput=False)
    bq_d = nc.declare_dram_parameter("bq", [128, HL], F32, isOutput=False)
    bk_d = nc.declare_dram_parameter("bk", [128, HL], F32, isOutput=False)
    out_ds = [
        nc.declare_dram_parameter(f"out{p}", [S, D], BF16, isOutput=True)
        for p in range(len(P3_PARTS))
    ]

    v_d = nc.dram_tensor("v_spill", [TB, 128, HL * DH], BF16)
    ct_d = nc.dram_tensor("ct_spill", [4, 128, S], BF16)  # heads 0-3 only
    den_d = nc.dram_tensor("den_bounce", [HL, 1, S], F32)
    rden_d = nc.dram_tensor("rden_bounce", [HL, 1, S], BF16)

    with tile.TileContext(nc) as tc:
        with (
            tc.tile_pool(name="xt_pool", bufs=1) as xt_pool,
            tc.tile_pool(name="const_pool", bufs=1) as const_pool,
            tc.tile_pool(name="w_pool", bufs=4) as w_pool,
            tc.tile_pool(name="qs_pool", bufs=2) as qs_pool,
            tc.tile_pool(name="qb_pool", bufs=4) as qb_pool,
            tc.tile_pool(name="v2_pool", bufs=2) as v2_pool,
            tc.tile_pool(name="et_pool", bufs=2) as et_pool,
            tc.tile_pool(name="pden_pool", bufs=2) as pden_pool,
            tc.tile_pool(name="den_pool", bufs=2) as den_pool,
            tc.tile_pool(name="norm_pool", bufs=2) as norm_pool,
            tc.tile_pool(name="psum", bufs=1, space="PSUM") as psum,
        ):
            # ---------------- prologue DMAs ----------------
            xt_sb = xt_pool.tile([128, KO, S], BF16)

            cos_sb = const_pool.tile([128, S], BF16)
            sin_sb = const_pool.tile([128, S], BF16)
            bq_sb = const_pool.tile([128, HL], F32)
            bk_sb = const_pool.tile([128, HL], F32)
            ones_sb = const_pool.tile([128, 1], BF16)
            warm_sb = const_pool.tile([128, 512], BF16)
            nc.vector.memset(ones_sb[:], 1.0)
            nc.vector.memset(warm_sb[:], 0.0)

            # PE warm-up: dummy matmuls with no data dependencies run while
            # the first input DMAs land, flipping the HAM clock gate to 8/8
            # before the first real matmul issues.
            for _ in range(12):
                wps = psum.tile([128, 512], F32, tag="sc", bufs=2, name="warm_ps")
                nc.tensor.matmul(
                    wps[:], warm_sb[:, 0:128], warm_sb[:], start=True, stop=True
                )

            # per-head weight tiles (ring of 4: two heads in flight)
            w_tiles = {}

            def dma_w(h):
                for nm, w_d in (("q", wq_d), ("k", wk_d)):
                    t = w_pool.tile([128, KO, 128], BF16, tag="w", bufs=4, name=f"w_{nm}")
                    nc.sync.dma_start(
                        out=t[:], in_=w_d[h].rearrange("k p m -> p k m")
                    )
                    w_tiles[(h, nm)] = t

            v2_tiles = {}

            def dma_v2(h):
                t = v2_pool.tile([128, TB, DH], BF16, name="v2")
                nc.sync.dma_start(
                    out=t[:],
                    in_=v_d[:, :, h * DH : (h + 1) * DH].rearrange("t p m -> p t m"),
                )
                v2_tiles[h] = t

            # qb tiles (rope'd q^T / k^T, bf16, ring of 4)
            qb_tiles = {}
            pden_tiles = {}
            cux_tiles = {}

            # ---------- filler piece machinery ----------
            def proj_pieces(h):
                """Q/K projection + rope for head h, as a list of closures."""
                pieces = []
                state = {}

                def chunk_first(nm, n, h=h):
                    def run():
                        w_sb = w_tiles[(h, nm)]
                        ps = psum.tile([128, 512], F32, tag="proj", bufs=2, name="proj_ps")
                        state[(nm, n)] = ps
                        for ko in range(8):
                            nc.tensor.matmul(
                                ps[:],
                                w_sb[:, ko, :],
                                xt_sb[:, ko, n * 512 : (n + 1) * 512],
                                start=(ko == 0),
                                stop=False,
                            )

                    return run

                def chunk_second(nm, n, h=h):
                    def run():
                        w_sb = w_tiles[(h, nm)]
                        ps = state[(nm, n)]
                        for ko in range(8, KO):
                            nc.tensor.matmul(
                                ps[:],
                                w_sb[:, ko, :],
                                xt_sb[:, ko, n * 512 : (n + 1) * 512],
                                start=False,
                                stop=(ko == KO - 1),
                            )
                        b_sb, scale = (
                            (bq_sb, QSCALE) if nm == "q" else (bk_sb, 1.0)
                        )
                        qs = state[("qs", nm)]
                        nc.vector.tensor_scalar(
                            out=qs[:, n * 512 : (n + 1) * 512],
                            in0=ps[:],
                            scalar1=scale,
                            scalar2=b_sb[:, h : h + 1],
                            op0=mybir.AluOpType.mult,
                            op1=mybir.AluOpType.add,
                        )

                    return run

                def rot_dma(nm):
                    # shuffle the halves straight into the qb tile (no
                    # separate rot buffer); rope then runs in place
                    def run():
                        qs = state[("qs", nm)]
                        qb = qb_pool.tile([128, S], BF16, tag="qb", bufs=4, name=f"qb_{nm}")
                        state[("qb", nm)] = qb
                        nc.sync.dma_start(out=qb[0:64, :], in_=qs[64:128, :])
                        nc.sync.dma_start(out=qb[64:128, :], in_=qs[0:64, :])

                    return run

                def rope(nm, h=h):
                    def run():
                        qs = state[("qs", nm)]
                        qb = state[("qb", nm)]
                        nc.vector.tensor_mul(out=qb[:], in0=qb[:], in1=sin_sb[:])
                        nc.vector.tensor_mul(out=qs[:], in0=qs[:], in1=cos_sb[:])
                        nc.vector.tensor_add(out=qb[:], in0=qb[:], in1=qs[:])
                        qb_tiles[(h, nm)] = qb

                    return run

                for nm in ("q", "k"):

                    def alloc_qs(nm=nm):
                        state[("qs", nm)] = qs_pool.tile([128, S], BF16, tag="qs", bufs=2, name=f"qs_{nm}")

                    pieces.append(alloc_qs)
                    for n in range(4):
                        pieces.append(chunk_first(nm, n))
                        pieces.append(chunk_second(nm, n))
                    pieces.append(rot_dma(nm))
                    pieces.append(rope(nm))
                return pieces

            def den_pieces(h):
                """Softmax denominator + ct normalize (and spill for h<4)."""
                pieces = []
                state = {}

                def den_mm():
                    pden = pden_tiles[h]
                    for n in range(4):
                        dps = psum.tile([1, 512], F32, tag="proj", bufs=2, name="dps")
                        nc.tensor.matmul(
                            dps[:],
                            ones_sb[:],
                            pden[:, n * 512 : (n + 1) * 512],
                            start=True,
                            stop=True,
                        )
                        dsb = den_pool.tile([1, 512], F32, tag="den", bufs=1, name="dsb")
                        nc.scalar.copy(out=dsb[:], in_=dps[:])
                        nc.sync.dma_start(
                            out=den_d[h][:, n * 512 : (n + 1) * 512], in_=dsb[:]
                        )

                def den_out():
                    # transposed read-back: [128, 16] so reciprocal is cheap
                    d128 = den_pool.tile([128, 16], F32, tag="d128", bufs=2, name="d128")
                    state["d128"] = d128
                    den_ap = den_d[h]
                    src = bass.AP(
                        tensor=den_ap.tensor,
                        offset=den_ap.offset,
                        ap=[[16, 128], [1, 16]],
                    )
                    nc.sync.dma_start(out=d128[:], in_=src)

                def recip():
                    d128 = state["d128"]
                    nc.vector.reciprocal(out=d128[:], in_=d128[:])
                    d128b = den_pool.tile([128, 16], BF16, tag="d128b", bufs=2, name="d128b")
                    nc.vector.tensor_copy(out=d128b[:], in_=d128[:])
                    rden_ap = rden_d[h]
                    dst = bass.AP(
                        tensor=rden_ap.tensor,
                        offset=rden_ap.offset,
                        ap=[[16, 128], [1, 16]],
                    )
                    nc.sync.dma_start(out=dst, in_=d128b[:])
                    # broadcast back across partitions
                    bc = norm_pool.tile([128, S], BF16, tag="bc", bufs=1, name="bc")
                    state["bc"] = bc
                    bsrc = bass.AP(
                        tensor=rden_ap.tensor,
                        offset=rden_ap.offset,
                        ap=[[0, 128]] + list(rden_ap.ap[1:]),
                    )
                    nc.sync.dma_start(out=bc[:], in_=bsrc)

                def norm():
                    cu = state["cu"]
                    bc = state["bc"]
                    if h < 4:
                        # normalize in place, then spill
                        nc.vector.tensor_mul(out=cu[:], in0=cu[:], in1=bc[:])
                        nc.sync.dma_start(out=ct_d[h], in_=cu[:])
                    else:
                        cux = cux_pool.tile([128, S], BF16, name="cux")
                        nc.vector.tensor_mul(out=cux[:], in0=cu[:], in1=bc[:])
                        cux_tiles[h] = cux

                pieces.append(den_mm)
                pieces.append(den_out)
                pieces.append(recip)
                pieces.append(norm)
                return pieces, state

            def p3_copy(ps, osb, n, mden=None):
                # alternate copy engine: vector for even n, scalar for odd
                sl = slice(n * 512, (n + 1) * 512)
                if n % 2 == 0:
                    nc.vector.tensor_copy(out=osb[:, sl], in_=ps[:])
                else:
                    nc.scalar.copy(out=osb[:, sl], in_=ps[:])

            def p3_pieces(part):
                """Output-projection groups for P3 partial `part`."""
                c0, c1 = P3_PARTS[part]
                ncs = c1 - c0
                use_sbuf = c0 >= 4
                pieces = []
                state = {}

                def cts_dma(m):
                    def run():
                        t = cts_pool.tile([128, ncs, 128], BF16, name="cts")
                        state[("cts", m)] = t
                        nc.sync.dma_start(
                            out=t[:],
                            in_=ct_d[c0:c1, :, m * 128 : (m + 1) * 128].rearrange(
                                "c p m2 -> p c m2"
                            ),
                        )

                    return run

                def alloc_osb(m):
                    def run():
                        state[("osb", m)] = osb_pool.tile([128, D], BF16, name="osb")

                    return run

                def group(m, n):
                    def run():
                        ps = psum.tile([128, 512], F32, tag="proj", bufs=2, name="p3_ps")
                        for i in range(ncs):
                            if use_sbuf:
                                lhsT = cux_tiles[c0 + i][:, m * 128 : (m + 1) * 128]
                            else:
                                lhsT = state[("cts", m)][:, i, :]
                            nc.tensor.matmul(
                                ps[:],
                                lhsT,
                                wo_sb[:, c0 + i, n * 512 : (n + 1) * 512],
                                start=(i == 0),
                                stop=(i == ncs - 1),
                            )
                        osb = state[("osb", m)]
                        p3_copy(ps, osb, n)
                        if n == 3:
                            nc.sync.dma_start(
                                out=out_ds[part][m * 128 : (m + 1) * 128, :],
                                in_=osb[:],
                            )

                    return run

                if not use_sbuf:
                    pieces.append(cts_dma(0))
                for m in range(TB):
                    pieces.append(alloc_osb(m))
                    if not use_sbuf and m + 1 < TB:
                        pieces.append(cts_dma(m + 1))
                    for n in range(4):
                        pieces.append(group(m, n))
                return pieces

            # ---------------- V phase (with head-0 projection as filler) ---
            with tc.tile_pool(name="wv_pool", bufs=1) as wv_pool, tc.tile_pool(
                name="vout_pool", bufs=3
            ) as vout_pool:
                wv_sb = wv_pool.tile([128, KO, HL * DH], BF16)

                def xt_dma(q):
                    sl = slice(q * 512, (q + 1) * 512)
                    nc.sync.dma_start(
                        out=xt_sb[:, :, sl],
                        in_=xt_d[:, :, sl].rearrange("k p s -> p k s"),
                    )

                def wv_dma(half):
                    # scalar DMA queue: streams in parallel with xt on sync
                    sl = slice(half * 512, (half + 1) * 512)
                    nc.scalar.dma_start(
                        out=wv_sb[:, :, sl],
                        in_=wv_d[:, :, sl].rearrange("k p m -> p k m"),
                    )

                # sync queue: xt quarters + q/k weights, ordered so the
                # in-order PE queue never stalls: head-0 w before xt_q1
                # (proj filler piece #1 needs it ~20us in), xt_q1 right
                # after (V units 4-7 need it ~30us in).  The scalar DMA
                # queue streams wv half 0, then the rope/bias constants
                # (needed ~25us in), then wv half 1 (needed ~75us in).
                xt_dma(0)
                wv_dma(0)
                dma_w(0)
                xt_dma(1)
                nc.scalar.dma_start(out=cos_sb[:], in_=cos_d[:])
                nc.scalar.dma_start(out=sin_sb[:], in_=sin_d[:])
                nc.scalar.dma_start(out=bq_sb[:], in_=bq_d[:])
                nc.scalar.dma_start(out=bk_sb[:], in_=bk_d[:])
                dma_w(1)
                wv_dma(1)
                xt_dma(2)
                xt_dma(3)

                vfill = proj_pieces(0)
                vi = 0
                for nf in range(2):
                    for tb in range(TB):
                        vps = psum.tile([128, 512], F32, tag="sc", bufs=2)
                        for ko in range(KO):
                            nc.tensor.matmul(
                                vps[:],
                                xt_sb[:, ko, tb * 128 : (tb + 1) * 128],
                                wv_sb[:, ko, nf * 512 : (nf + 1) * 512],
                                start=(ko == 0),
                                stop=(ko == KO - 1),
                            )
                        vsb = vout_pool.tile([128, 512], BF16)
                        nc.vector.tensor_copy(out=vsb[:], in_=vps[:])
                        nc.sync.dma_start(
                            out=v_d[tb, :, nf * 512 : (nf + 1) * 512], in_=vsb[:]
                        )
                        # ~20 filler pieces over 32 V units
                        unit = nf * TB + tb
                        want = ((unit + 1) * len(vfill)) // 32
                        while vi < want:
                            vfill[vi]()
                            vi += 1
                        if nf == 0 and tb == TB - 1:
                            dma_v2(0)
                while vi < len(vfill):
                    vfill[vi]()
                    vi += 1

            # ---------------- head loop ----------------
            with (
                tc.tile_pool(name="wo_pool", bufs=1) as wo_pool,
                tc.tile_pool(name="cts_pool", bufs=2) as cts_pool,
                tc.tile_pool(name="osb_pool", bufs=2) as osb_pool,
                tc.tile_pool(name="cux_pool", bufs=4) as cux_pool,
            ):
              wo_sb = wo_pool.tile([128, HL, D], BF16)
              for n in range(4):
                sl = slice(n * 512, (n + 1) * 512)
                nc.scalar.dma_start(
                    out=wo_sb[:, :, sl],
                    in_=wo_d[:, :, sl].rearrange("c p m -> p c m"),
                )
              den_states = {}
              p3_p0 = p3_pieces(0)
              p0_cuts = [0, len(p3_p0) // 3, 2 * len(p3_p0) // 3, len(p3_p0)]
              p3_p1 = None
              p1_li = 0
              for h in range(HL):
                  # filler: early pieces (weight DMAs, prev head's softmax
                  # denominator, next head's projection+rope) must finish by
                  # slot 12 so the next head's attention starts stall-free;
                  # late pieces (P3 groups) spread over slots 4..15.
                  fill = []
                  fill_late = []
                  if h >= 1:
                      dp, _st = den_states[h - 1]
                      fill.extend(dp)
                  if h + 2 < HL:
                      fill.append(lambda hh=h + 2: dma_w(hh))
                  if h + 1 < HL:
                      fill.append(lambda hh=h + 1: dma_v2(hh))
                      fill.extend(proj_pieces(h + 1))
                  if h in (4, 5, 6):
                      # spread partial-0 groups over heads 4-6
                      fill_late.extend(p3_p0[p0_cuts[h - 4] : p0_cuts[h - 3]])
                  if h == 7:
                      # partial 1 (heads 4-6): most of it fills head 7; the
                      # remainder overlaps the tail's den(7) DMA chain
                      p3_p1 = p3_pieces(1)
                      fill_late.extend(p3_p1[: (len(p3_p1) * 13) // 16])

                  qt = qb_tiles[(h, "q")]
                  kt = qb_tiles[(h, "k")]
                  v2_sb = v2_tiles[h]

                  ctx_ps = psum.tile([128, S], F32, tag="ctx", bufs=1)
                  pden = pden_pool.tile([128, S], BF16)
                  pden_tiles[h] = pden

                  fi = 0
                  li = 0
                  for tb in range(TB):
                      et = et_pool.tile([128, S], BF16)
                      for n in range(4):
                          sc = psum.tile([128, 512], F32, tag="sc", bufs=2)
                          nc.tensor.matmul(
                              sc[:],
                              kt[:, tb * 128 : (tb + 1) * 128],
                              qt[:, n * 512 : (n + 1) * 512],
                              start=True,
                              stop=True,
                          )
                          nc.scalar.activation(
                              out=et[:, n * 512 : (n + 1) * 512],
                              in_=sc[:],
                              func=mybir.ActivationFunctionType.Exp,
                          )
                      # filler between scores and ctx.  Head 7's fill is just
                      # den(6), and partial 1 (which reads cux6 produced by
                      # its norm piece) starts at slot 4 -- so ramp it in 3
                      # slots there instead of 12.
                      ramp = 3 if h == 7 else 12
                      want = min(len(fill), ((tb + 1) * len(fill) + ramp - 1) // ramp)
                      while fi < want:
                          fill[fi]()
                          fi += 1
                      # fill_late starts at slot 4: the first P3 group of a
                      # partial reads ct written by den pieces that are
                      # emitted in this head's slots 0-2
                      if tb >= 4:
                          want_l = ((tb - 3) * len(fill_late)) // (TB - 4)
                          while li < want_l:
                              fill_late[li]()
                              li += 1
                      for n in range(4):
                          nc.tensor.matmul(
                              ctx_ps[:, n * 512 : (n + 1) * 512],
                              v2_sb[:, tb, :],
                              et[:, n * 512 : (n + 1) * 512],
                              start=(tb == 0),
                              stop=(tb == TB - 1),
                          )
                      if tb == 0:
                          nc.vector.tensor_copy(out=pden[:], in_=et[:])
                      else:
                          nc.vector.tensor_add(out=pden[:], in0=pden[:], in1=et[:])
                  if h == 7:
                      p1_li = li

                  # free ctx PSUM quickly: unnormalized ctx^T to SBUF (bf16),
                  # in 4 chunks so banks release progressively
                  dp, st = den_pieces(h)
                  den_states[h] = (dp, st)
                  cu = norm_pool.tile([128, S], BF16, tag="cu", bufs=1)
                  for n in range(4):
                      sl = slice(n * 512, (n + 1) * 512)
                      nc.scalar.copy(out=cu[:, sl], in_=ctx_ps[:, sl])
                  st["cu"] = cu

              # ---------------- tail ----------------
              # den(7) chain first (starts its serial DMA chain ASAP), then
              # the leftover partial-1 pieces keep the PE busy under it, then
              # partial 2 (head 7) which waits on the normalized cux7.
              for piece in den_states[7][0]:
                  piece()
              while p1_li < len(p3_p1):
                  p3_p1[p1_li]()
                  p1_li += 1
              for piece in p3_pieces(2):
                  piece()

    nc.finalize()
    return nc


def _get_nc():
    if "nc" not in _NC_CACHE:
        _NC_CACHE["nc"] = build_nc()
    return _NC_CACHE["nc"]


def _rope_tables():
    inv_freq = 1.0 / (ROPE_THETA ** (np.arange(0, DH, 2, dtype=np.float32) / DH))
    freqs = np.arange(S, dtype=np.float32)[:, None] * inv_freq[None, :]
    emb = np.concatenate([freqs, freqs], axis=-1)  # [S, 128]
    cosT = np.ascontiguousarray(np.cos(emb).T.astype(NPBF))  # [128, S]
    sinS = np.sin(emb).T.astype(np.float32).copy()
    sinS[0:64, :] *= -1.0  # sign-folded rotate_half
    return cosT, np.ascontiguousarray(sinS.astype(NPBF))


def kernel(x, wq, bq, wk, bk, wv, bv, wo, bo, _trace=False, _tmpdir=None):
    x = np.asarray(x, dtype=np.float32)
    wq = np.asarray(wq, dtype=np.float32)
    wk = np.asarray(wk, dtype=np.float32)
    wv = np.asarray(wv, dtype=np.float32)
    wo = np.asarray(wo, dtype=np.float32)
    bq = np.asarray(bq, dtype=np.float32)
    bk = np.asarray(bk, dtype=np.float32)
    bv = np.asarray(bv, dtype=np.float32)
    bo = np.asarray(bo, dtype=np.float32)

    nc = _get_nc()
    cosT, sinS = _rope_tables()

    def qk_pack(w, g):
        ws = w[g * 1024 : (g + 1) * 1024, :]
        return np.ascontiguousarray(
            ws.reshape(HL, 128, KO, 128).transpose(0, 2, 3, 1).astype(NPBF)
        )

    packs = []
    for g in range(2):
        wv_s = wv[g * 1024 : (g + 1) * 1024, :]
        wv_p = np.ascontiguousarray(
            wv_s.reshape(HL * DH, KO, 128).transpose(1, 2, 0).astype(NPBF)
        )
        wo_s = wo[:, g * 1024 : (g + 1) * 1024]
        wo_p = np.ascontiguousarray(
            wo_s.reshape(D, HL, 128).transpose(1, 2, 0).astype(NPBF)
        )
        bq_p = np.ascontiguousarray(
            (bq[g * 1024 : (g + 1) * 1024] * QSCALE).reshape(HL, 128).T
        )
        bk_p = np.ascontiguousarray(bk[g * 1024 : (g + 1) * 1024].reshape(HL, 128).T)
        packs.append(
            dict(
                wq=qk_pack(wq, g),
                wk=qk_pack(wk, g),
                wv=wv_p,
                wo=wo_p,
                bq=bq_p,
                bk=bk_p,
            )
        )

    in_maps = []
    xts = [
        np.ascontiguousarray(x[b].T.astype(NPBF)).reshape(KO, 128, S)
        for b in range(B)
    ]
    for c in range(8):
        b, g = c // 2, c % 2
        m = dict(packs[g])
        m["xt"] = xts[b]
        m["cosT"] = cosT
        m["sinS"] = sinS
        in_maps.append(m)

    res = run_bass_kernel_spmd(
        nc,
        in_maps,
        core_ids=list(range(8)),
        trace=_trace,
        tmpdir=_tmpdir,
    )

    bo_eff = bo + wo @ bv
    out = np.empty((B, S, D), dtype=np.float32)
    for b in range(B):
        acc = np.zeros((S, D), dtype=np.float32)
        for c in (2 * b, 2 * b + 1):
            for p in range(len(P3_PARTS)):
                acc += res.results[c][f"out{p}"].astype(np.float32)
        out[b] = acc + bo_eff[None, :]
    if _trace:
        kernel.last_result = res
    return out


# revision 20
# speedup vs baseline: 1.1053x; 1.0672x over previous
"""Multi-head attention (RoPE, non-causal) on 8 Trainium2 NeuronCores.

Problem: x[4,2048,2048] fp32; wq/wk/wv/wo [2048,2048]; biases [2048].
  q,k,v = x@w.T+b per 16 heads of dim 128; rope(q,k); softmax(q k^T/sqrt(128));
  out = (attn@v)@wo.T + bo.

Sharding: core c = 2*b + g -> batch b, head-group g (8 heads each).
Each core computes a partial output (its 8 heads) for its batch over the full
sequence; the host sums the partials (the wo contraction splits cleanly over
head groups) and adds bo_eff = bo + wo@bv (the V bias folds out exactly
because softmax rows sum to 1).

v3 (tail + engine-balance rework over v2's software pipeline):
  * P3 partials regrouped to [(0,4),(4,7),(7,8)]: partial 0 (heads 0-3,
    DRAM-bounced ct) fills heads 4-6; partial 1 (heads 4-6, normalized ct
    kept in SBUF - no spill/reload) fills head 7; partial 2 (head 7) runs
    in the tail, its matmuls emitted after the leftover partial-1 pieces so
    the PE stays busy under head 7's softmax-denominator DMA chain.
  * P3 PSUM->SBUF copies alternate between the Vector and Scalar engines
    (v2 put all of them on Vector, which serialized the tail at CAST speed).
  * Startup: wv streams on the Scalar DMA queue in parallel with xt on the
    Sync queue, and a burst of dummy matmuls warms the PE HAM clock while
    the first 4MB lands, so the first real matmul runs at 2.4GHz ~14us in.
  * The unnormalized-ctx PSUM->SBUF copy is split in 4 so ctx PSUM banks
    free progressively for the next head's first ctx matmuls.
"""

import sys

if "/opt/trn_rl_repo" not in sys.path:
    sys.path.insert(0, "/opt/trn_rl_repo")

import ml_dtypes
import numpy as np

import concourse.bass as bass
import concourse.tile as tile
from concourse import bacc, mybir
from concourse.bass_utils import run_bass_kernel_spmd

F32 = mybir.dt.float32
BF16 = mybir.dt.bfloat16
NPBF = ml_dtypes.bfloat16

B, S, D = 4, 2048, 2048
H = 16
DH = 128
HL = 8  # heads per core
KO = D // 128  # 16 k-chunks
TB = S // 128  # 16 t-chunks
ROPE_THETA = 10000.0
QSCALE = 1.0 / np.sqrt(DH)

# P3 partial-output head ranges:
#   partial 0: heads 0-3 (ct bounced via DRAM), filler during heads 4-6
#   partial 1: heads 4-6 (ct in SBUF), filler during head 7 + tail overlap
#   partial 2: head 7 (ct in SBUF), tail
P3_PARTS = [(0, 4), (4, 7), (7, 8)]

_NC_CACHE = {}


def build_nc():
    nc = bacc.Bacc()

    xt_d = nc.declare_dram_parameter("xt", [KO, 128, S], BF16, isOutput=False)
    wq_d = nc.declare_dram_parameter("wq", [HL, KO, 128, 128], BF16, isOutput=False)
    wk_d = nc.declare_dram_parameter("wk", [HL, KO, 128, 128], BF16, isOutput=False)
    wv_d = nc.declare_dram_parameter("wv", [KO, 128, HL * DH], BF16, isOutput=False)
    wo_d = nc.declare_dram_parameter("wo", [HL, 128, D], BF16, isOutput=False)
    cos_d = nc.declare_dram_parameter("cosT", [128, S], BF16, isOutput=False)
    sin_d = nc.declare_dram_parameter("sinS", [128, S], BF16, isOutput=False)
    bq_d = nc.declare_dram_parameter("bq", [128, HL], F32, isOutput=False)
    bk_d = nc.declare_dram_parameter("bk", [128, HL], F32, isOutput=False)
    out_ds = [
        nc.declare_dram_parameter(f"out{p}", [S, D], BF16, isOutput=True)
        for p in range(len(P3_PARTS))
    ]

    v_d = nc.dram_tensor("v_spill", [TB, 128, HL * DH], BF16)
    ct_d = nc.dram_tensor("ct_spill", [4, 128, S], BF16)  # heads 0-3 only
    den_d = nc.dram_tensor("den_bounce", [HL, 1, S], F32)
    rden_d = nc.dram_tensor("rden_bounce", [HL, 1, S], BF16)

    with tile.TileContext(nc) as tc:
        with (
            tc.tile_pool(name="xt_pool", bufs=1) as xt_pool,
            tc.tile_pool(name="const_pool", bufs=1) as const_pool,
            tc.tile_pool(name="w_pool", bufs=4) as w_pool,
            tc.tile_pool(name="qs_pool", bufs=2) as qs_pool,
            tc.tile_pool(name="qb_pool", bufs=4) as qb_pool,
            tc.tile_pool(name="v2_pool", bufs=2) as v2_pool,
            tc.tile_pool(name="et_pool", bufs=2) as et_pool,
            tc.tile_pool(name="pden_pool", bufs=2) as pden_pool,
            tc.tile_pool(name="den_pool", bufs=2) as den_pool,
            tc.tile_pool(name="norm_pool", bufs=2) as norm_pool,
            tc.tile_pool(name="psum", bufs=1, space="PSUM") as psum,
        ):
            # ---------------- prologue DMAs ----------------
            xt_sb = xt_pool.tile([128, KO, S], BF16)

            cos_sb = const_pool.tile([128, S], BF16)
            sin_sb = const_pool.tile([128, S], BF16)
            bq_sb = const_pool.tile([128, HL], F32)
            bk_sb = const_pool.tile([128, HL], F32)
            ones_sb = const_pool.tile([128, 1], BF16)
            warm_sb = const_pool.tile([128, 512], BF16)
            nc.vector.memset(ones_sb[:], 1.0)
            nc.vector.memset(warm_sb[:], 0.0)

            # PE warm-up: dummy matmuls with no data dependencies run while
            # the first input DMAs land (~21us for xt_q0+wv0 on two queues),
            # flipping the HAM clock gate to 8/8 before the first real
            # matmul issues and keeping it there.
            for _ in range(44):
                wps = psum.tile([128, 512], F32, tag="sc", bufs=2, name="warm_ps")
                nc.tensor.matmul(
                    wps[:], warm_sb[:, 0:128], warm_sb[:], start=True, stop=True
                )

            # per-head weight tiles (ring of 4: two heads in flight)
            w_tiles = {}

            def dma_w(h, queue=None):
                q = queue if queue is not None else nc.sync
                for nm, w_d in (("q", wq_d), ("k", wk_d)):
                    t = w_pool.tile([128, KO, 128], BF16, tag="w", bufs=4, name=f"w_{nm}")
                    q.dma_start(
                        out=t[:], in_=w_d[h].rearrange("k p m -> p k m")
                    )
                    w_tiles[(h, nm)] = t

            v2_tiles = {}

            def dma_v2(h):
                t = v2_pool.tile([128, TB, DH], BF16, name="v2")
                nc.sync.dma_start(
                    out=t[:],
                    in_=v_d[:, :, h * DH : (h + 1) * DH].rearrange("t p m -> p t m"),
                )
                v2_tiles[h] = t

            # qb tiles (rope'd q^T / k^T, bf16, ring of 4)
            qb_tiles = {}
            pden_tiles = {}
            cux_tiles = {}

            # ---------- filler piece machinery ----------
            def proj_pieces(h):
                """Q/K projection + rope for head h, as a list of closures."""
                pieces = []
                state = {}

                def chunk_first(nm, n, h=h):
                    def run():
                        w_sb = w_tiles[(h, nm)]
                        ps = psum.tile([128, 512], F32, tag="proj", bufs=2, name="proj_ps")
                        state[(nm, n)] = ps
                        for ko in range(8):
                            nc.tensor.matmul(
                                ps[:],
                                w_sb[:, ko, :],
                                xt_sb[:, ko, n * 512 : (n + 1) * 512],
                                start=(ko == 0),
                                stop=False,
                            )

                    return run

                def chunk_second(nm, n, h=h):
                    def run():
                        w_sb = w_tiles[(h, nm)]
                        ps = state[(nm, n)]
                        for ko in range(8, KO):
                            nc.tensor.matmul(
                                ps[:],
                                w_sb[:, ko, :],
                                xt_sb[:, ko, n * 512 : (n + 1) * 512],
                                start=False,
                                stop=(ko == KO - 1),
                            )
                        b_sb, scale = (
                            (bq_sb, QSCALE) if nm == "q" else (bk_sb, 1.0)
                        )
                        qs = state[("qs", nm)]
                        nc.vector.tensor_scalar(
                            out=qs[:, n * 512 : (n + 1) * 512],
                            in0=ps[:],
                            scalar1=scale,
                            scalar2=b_sb[:, h : h + 1],
                            op0=mybir.AluOpType.mult,
                            op1=mybir.AluOpType.add,
                        )

                    return run

                def rot_dma(nm):
                    # shuffle the halves straight into the qb tile (no
                    # separate rot buffer); rope then runs in place
                    def run():
                        qs = state[("qs", nm)]
                        qb = qb_pool.tile([128, S], BF16, tag="qb", bufs=4, name=f"qb_{nm}")
                        state[("qb", nm)] = qb
                        nc.sync.dma_start(out=qb[0:64, :], in_=qs[64:128, :])
                        nc.sync.dma_start(out=qb[64:128, :], in_=qs[0:64, :])

                    return run

                def rope(nm, h=h):
                    def run():
                        qs = state[("qs", nm)]
                        qb = state[("qb", nm)]
                        nc.vector.tensor_mul(out=qb[:], in0=qb[:], in1=sin_sb[:])
                        nc.vector.tensor_mul(out=qs[:], in0=qs[:], in1=cos_sb[:])
                        nc.vector.tensor_add(out=qb[:], in0=qb[:], in1=qs[:])
                        qb_tiles[(h, nm)] = qb

                    return run

                for nm in ("q", "k"):

                    def alloc_qs(nm=nm):
                        state[("qs", nm)] = qs_pool.tile([128, S], BF16, tag="qs", bufs=2, name=f"qs_{nm}")

                    pieces.append(alloc_qs)
                    for n in range(4):
                        pieces.append(chunk_first(nm, n))
                        pieces.append(chunk_second(nm, n))
                    pieces.append(rot_dma(nm))
                    pieces.append(rope(nm))
                return pieces

            def den_pieces(h):
                """Softmax denominator + ct normalize (and spill for h<4).

                For head 7 (the tail) the chain is latency-critical: the
                den psum tiles alternate sc/proj tags (sc is idle there),
                and the broadcast + normalize are split in 4 chunks so
                partial-2 matmuls unlock per 512-column group.
                """
                pieces = []
                state = {}
                tail = h == 7

                def den_mm():
                    pden = pden_tiles[h]
                    for n in range(4):
                        tag = ("sc" if n % 2 else "proj") if tail else "proj"
                        dps = psum.tile([1, 512], F32, tag=tag, bufs=2, name="dps")
                        nc.tensor.matmul(
                            dps[:],
                            ones_sb[:],
                            pden[:, n * 512 : (n + 1) * 512],
                            start=True,
                            stop=True,
                        )
                        dsb = den_pool.tile([1, 512], F32, tag="den", bufs=2, name="dsb")
                        nc.vector.tensor_copy(out=dsb[:], in_=dps[:])
                        nc.sync.dma_start(
                            out=den_d[h][:, n * 512 : (n + 1) * 512], in_=dsb[:]
                        )

                def den_out():
                    # transposed read-back: [128, 16] so reciprocal is cheap
                    d128 = den_pool.tile([128, 16], F32, tag="d128", bufs=2, name="d128")
                    state["d128"] = d128
                    den_ap = den_d[h]
                    src = bass.AP(
                        tensor=den_ap.tensor,
                        offset=den_ap.offset,
                        ap=[[16, 128], [1, 16]],
                    )
                    nc.sync.dma_start(out=d128[:], in_=src)

                def recip():
                    d128 = state["d128"]
                    nc.vector.reciprocal(out=d128[:], in_=d128[:])
                    d128b = den_pool.tile([128, 16], BF16, tag="d128b", bufs=2, name="d128b")
                    nc.vector.tensor_copy(out=d128b[:], in_=d128[:])
                    rden_ap = rden_d[h]
                    dst = bass.AP(
                        tensor=rden_ap.tensor,
                        offset=rden_ap.offset,
                        ap=[[16, 128], [1, 16]],
                    )
                    nc.sync.dma_start(out=dst, in_=d128b[:])
                    # broadcast back across partitions
                    bc = norm_pool.tile([128, S], BF16, tag="bc", bufs=1, name="bc")
                    state["bc"] = bc
                    rt, ro = rden_ap.tensor, rden_ap.offset
                    if tail:
                        for n in range(4):
                            bsrc = bass.AP(
                                tensor=rt, offset=ro + n * 512,
                                ap=[[0, 128], [1, 512]],
                            )
                            nc.sync.dma_start(
                                out=bc[:, n * 512 : (n + 1) * 512], in_=bsrc
                            )
                    else:
                        bsrc = bass.AP(
                            tensor=rt, offset=ro,
                            ap=[[0, 128]] + list(rden_ap.ap[1:]),
                        )
                        nc.sync.dma_start(out=bc[:], in_=bsrc)

                def norm():
                    cu = state["cu"]
                    bc = state["bc"]
                    if h < 4:
                        # normalize in place, then spill
                        nc.vector.tensor_mul(out=cu[:], in0=cu[:], in1=bc[:])
                        nc.sync.dma_start(out=ct_d[h], in_=cu[:])
                    else:
                        cux = cux_pool.tile([128, S], BF16, name="cux")
                        if tail:
                            for n in range(4):
                                sl = slice(n * 512, (n + 1) * 512)
                                nc.vector.tensor_mul(
                                    out=cux[:, sl], in0=cu[:, sl], in1=bc[:, sl]
                                )
                        else:
                            nc.vector.tensor_mul(out=cux[:], in0=cu[:], in1=bc[:])
                        cux_tiles[h] = cux

                pieces.append(den_mm)
                pieces.append(den_out)
                pieces.append(recip)
                pieces.append(norm)
                return pieces, state

            def p3_copy(ps, osb, n):
                # alternate copy engine: vector for even n, scalar for odd
                sl = slice(n * 512, (n + 1) * 512)
                if n % 2 == 0:
                    nc.vector.tensor_copy(out=osb[:, sl], in_=ps[:])
                else:
                    nc.scalar.copy(out=osb[:, sl], in_=ps[:])

            # In the tail the sc psum tag is idle, so P3 groups alternate
            # sc/proj tags there: 4 banks in flight instead of 2, which
            # decouples the MM -> copy -> MM serialization.
            p3_alt = {"on": False, "i": 0}

            def p3_pieces(part):
                """Output-projection groups for P3 partial `part`."""
                c0, c1 = P3_PARTS[part]
                ncs = c1 - c0
                use_sbuf = c0 >= 4
                pieces = []
                state = {}

                def cts_dma(m):
                    def run():
                        t = cts_pool.tile([128, ncs, 128], BF16, name="cts")
                        state[("cts", m)] = t
                        nc.sync.dma_start(
                            out=t[:],
                            in_=ct_d[c0:c1, :, m * 128 : (m + 1) * 128].rearrange(
                                "c p m2 -> p c m2"
                            ),
                        )

                    return run

                def alloc_osb(m):
                    def run():
                        state[("osb", m)] = osb_pool.tile([128, D], BF16, name="osb")

                    return run

                def group(m, n):
                    def run():
                        if p3_alt["on"]:
                            p3_alt["i"] += 1
                            tag = "sc" if p3_alt["i"] % 2 else "proj"
                        else:
                            tag = "proj"
                        ps = psum.tile([128, 512], F32, tag=tag, bufs=2, name="p3_ps")
                        for i in range(ncs):
                            if use_sbuf:
                                lhsT = cux_tiles[c0 + i][:, m * 128 : (m + 1) * 128]
                            else:
                                lhsT = state[("cts", m)][:, i, :]
                            nc.tensor.matmul(
                                ps[:],
                                lhsT,
                                wo_sb[:, c0 + i, n * 512 : (n + 1) * 512],
                                start=(i == 0),
                                stop=(i == ncs - 1),
                            )
                        osb = state[("osb", m)]
                        p3_copy(ps, osb, n)
                        if n == 3:
                            nc.sync.dma_start(
                                out=out_ds[part][m * 128 : (m + 1) * 128, :],
                                in_=osb[:],
                            )

                    return run

                if not use_sbuf:
                    pieces.append(cts_dma(0))
                for m in range(TB):
                    pieces.append(alloc_osb(m))
                    if not use_sbuf and m + 1 < TB:
                        pieces.append(cts_dma(m + 1))
                    for n in range(4):
                        pieces.append(group(m, n))
                return pieces

            # ---------------- V phase (with head-0 projection as filler) ---
            with tc.tile_pool(name="wv_pool", bufs=1) as wv_pool, tc.tile_pool(
                name="vout_pool", bufs=3
            ) as vout_pool:
                wv_sb = wv_pool.tile([128, KO, HL * DH], BF16)

                def xt_dma(q):
                    sl = slice(q * 512, (q + 1) * 512)
                    nc.sync.dma_start(
                        out=xt_sb[:, :, sl],
                        in_=xt_d[:, :, sl].rearrange("k p s -> p k s"),
                    )

                def wv_dma(half):
                    # scalar DMA queue: streams in parallel with xt on sync
                    sl = slice(half * 512, (half + 1) * 512)
                    nc.scalar.dma_start(
                        out=wv_sb[:, :, sl],
                        in_=wv_d[:, :, sl].rearrange("k p m -> p k m"),
                    )

                # The sync DMA queue (~190GB/s) carries ONLY the xt quarters
                # so V units are never starved (tb4/8/12 need q1/q2/q3 at
                # ~40/57/74us; they land ~30/40/51us).  Everything else for
                # the V phase + head 0/1 rides the scalar queue: wv half 0
                # (first V matmul), head-0 q/k weights (proj filler ~27us),
                # rope/bias constants (~31us), head-1 weights, wv half 1
                # (V unit 16, ~91us).
                xt_dma(0)
                wv_dma(0)
                dma_w(0, queue=nc.scalar)
                xt_dma(1)
                nc.scalar.dma_start(out=cos_sb[:], in_=cos_d[:])
                nc.scalar.dma_start(out=sin_sb[:], in_=sin_d[:])
                nc.scalar.dma_start(out=bq_sb[:], in_=bq_d[:])
                nc.scalar.dma_start(out=bk_sb[:], in_=bk_d[:])
                dma_w(1, queue=nc.scalar)
                wv_dma(1)
                xt_dma(2)
                xt_dma(3)

                vfill = proj_pieces(0)
                vi = 0
                for nf in range(2):
                    for tb in range(TB):
                        vps = psum.tile([128, 512], F32, tag="sc", bufs=2)
                        for ko in range(KO):
                            nc.tensor.matmul(
                                vps[:],
                                xt_sb[:, ko, tb * 128 : (tb + 1) * 128],
                                wv_sb[:, ko, nf * 512 : (nf + 1) * 512],
                                start=(ko == 0),
                                stop=(ko == KO - 1),
                            )
                        vsb = vout_pool.tile([128, 512], BF16)
                        nc.vector.tensor_copy(out=vsb[:], in_=vps[:])
                        nc.sync.dma_start(
                            out=v_d[tb, :, nf * 512 : (nf + 1) * 512], in_=vsb[:]
                        )
                        # ~20 filler pieces over 32 V units
                        unit = nf * TB + tb
                        want = ((unit + 1) * len(vfill)) // 32
                        while vi < want:
                            vfill[vi]()
                            vi += 1
                        if nf == 0 and tb == TB - 1:
                            dma_v2(0)
                while vi < len(vfill):
                    vfill[vi]()
                    vi += 1

            # ---------------- head loop ----------------
            with (
                tc.tile_pool(name="wo_pool", bufs=1) as wo_pool,
                tc.tile_pool(name="cts_pool", bufs=2) as cts_pool,
                tc.tile_pool(name="osb_pool", bufs=2) as osb_pool,
                tc.tile_pool(name="cux_pool", bufs=4) as cux_pool,
            ):
              wo_sb = wo_pool.tile([128, HL, D], BF16)
              for n in range(4):
                sl = slice(n * 512, (n + 1) * 512)
                nc.scalar.dma_start(
                    out=wo_sb[:, :, sl],
                    in_=wo_d[:, :, sl].rearrange("c p m -> p c m"),
                )
              den_states = {}
              p3_p0 = p3_pieces(0)
              p0_cuts = [0, len(p3_p0) // 3, 2 * len(p3_p0) // 3, len(p3_p0)]
              p3_p1 = None
              p1_li = 0
              for h in range(HL):
                  # filler: den(h-1) pieces interleave with the next head's
                  # projection chunks (so early slots stay matmul-dense
                  # instead of exp-bound), the whole list paced over 14 of
                  # the 16 slots; late pieces (P3 groups) spread over slots
                  # 4..15.
                  den_p = []
                  rest = []
                  fill_late = []
                  if h >= 1:
                      dp, _st = den_states[h - 1]
                      den_p = list(dp)
                  if h + 1 < HL:
                      rest.extend(proj_pieces(h + 1))
                      rest.insert(8, lambda hh=h + 1: dma_v2(hh))
                      if h + 2 < HL:
                          rest.insert(6, lambda hh=h + 2: dma_w(hh))
                  fill = []
                  for piece in den_p:
                      fill.append(piece)
                      if rest:
                          fill.append(rest.pop(0))
                  fill.extend(rest)
                  if h in (4, 5, 6):
                      # spread partial-0 groups over heads 4-6
                      fill_late.extend(p3_p0[p0_cuts[h - 4] : p0_cuts[h - 3]])
                  if h == 7:
                      # partial 1 (heads 4-6): ~10/16 fills head 7; the
                      # remainder overlaps the tail's den(7) DMA chain
                      p3_p1 = p3_pieces(1)
                      fill_late.extend(p3_p1[: (len(p3_p1) * 10) // 16])

                  qt = qb_tiles[(h, "q")]
                  kt = qb_tiles[(h, "k")]
                  v2_sb = v2_tiles[h]

                  ctx_ps = psum.tile([128, S], F32, tag="ctx", bufs=1)
                  pden = pden_pool.tile([128, S], BF16)
                  pden_tiles[h] = pden

                  fi = 0
                  li = 0
                  for tb in range(TB):
                      et = et_pool.tile([128, S], BF16)
                      for n in range(4):
                          sc = psum.tile([128, 512], F32, tag="sc", bufs=2)
                          nc.tensor.matmul(
                              sc[:],
                              kt[:, tb * 128 : (tb + 1) * 128],
                              qt[:, n * 512 : (n + 1) * 512],
                              start=True,
                              stop=True,
                          )
                          nc.scalar.activation(
                              out=et[:, n * 512 : (n + 1) * 512],
                              in_=sc[:],
                              func=mybir.ActivationFunctionType.Exp,
                          )
                      # filler between scores and ctx.  Head 7's fill is just
                      # den(6), and partial 1 (which reads cux6 produced by
                      # its norm piece) starts at slot 4 -- so ramp it in 3
                      # slots there instead of 14.
                      ramp = 3 if h == 7 else 14
                      want = min(len(fill), ((tb + 1) * len(fill) + ramp - 1) // ramp)
                      while fi < want:
                          fill[fi]()
                          fi += 1
                      # fill_late starts at slot 4: the first P3 group of a
                      # partial reads ct written by den pieces that are
                      # emitted in this head's slots 0-2
                      if tb >= 4:
                          want_l = ((tb - 3) * len(fill_late)) // (TB - 4)
                          while li < want_l:
                              fill_late[li]()
                              li += 1
                      for n in range(4):
                          nc.tensor.matmul(
                              ctx_ps[:, n * 512 : (n + 1) * 512],
                              v2_sb[:, tb, :],
                              et[:, n * 512 : (n + 1) * 512],
                              start=(tb == 0),
                              stop=(tb == TB - 1),
                          )
                      if tb == 0:
                          nc.vector.tensor_copy(out=pden[:], in_=et[:])
                      else:
                          nc.vector.tensor_add(out=pden[:], in0=pden[:], in1=et[:])
                  if h == 7:
                      p1_li = li

                  # free ctx PSUM quickly: unnormalized ctx^T to SBUF (bf16),
                  # in 4 chunks so banks release progressively.  On VECTOR:
                  # the scalar queue must stay clear for the next head's
                  # slot-0 exps (in-order queue).
                  dp, st = den_pieces(h)
                  den_states[h] = (dp, st)
                  cu = norm_pool.tile([128, S], BF16, tag="cu", bufs=1)
                  for n in range(4):
                      sl = slice(n * 512, (n + 1) * 512)
                      nc.vector.tensor_copy(out=cu[:, sl], in_=ctx_ps[:, sl])
                  st["cu"] = cu

              # ---------------- tail ----------------
              # den(7) chain first (starts its serial DMA chain ASAP), then
              # the leftover partial-1 pieces keep the PE busy under it, then
              # partial 2 (head 7) which waits on the normalized cux7.  P3
              # psum tiles alternate sc/proj tags here (sc is free).
              p3_alt["on"] = True
              for piece in den_states[7][0]:
                  piece()
              while p1_li < len(p3_p1):
                  p3_p1[p1_li]()
                  p1_li += 1
              for piece in p3_pieces(2):
                  piece()

    nc.finalize()
    return nc


def _get_nc():
    if "nc" not in _NC_CACHE:
        _NC_CACHE["nc"] = build_nc()
    return _NC_CACHE["nc"]


def _rope_tables():
    inv_freq = 1.0 / (ROPE_THETA ** (np.arange(0, DH, 2, dtype=np.float32) / DH))
    freqs = np.arange(S, dtype=np.float32)[:, None] * inv_freq[None, :]
    emb = np.concatenate([freqs, freqs], axis=-1)  # [S, 128]
    cosT = np.ascontiguousarray(np.cos(emb).T.astype(NPBF))  # [128, S]
    sinS = np.sin(emb).T.astype(np.float32).copy()
    sinS[0:64, :] *= -1.0  # sign-folded rotate_half
    return cosT, np.ascontiguousarray(sinS.astype(NPBF))


def kernel(x, wq, bq, wk, bk, wv, bv, wo, bo, _trace=False, _tmpdir=None):
    x = np.asarray(x, dtype=np.float32)
    wq = np.asarray(wq, dtype=np.float32)
    wk = np.asarray(wk, dtype=np.float32)
    wv = np.asarray(wv, dtype=np.float32)
    wo = np.asarray(wo, dtype=np.float32)
    bq = np.asarray(bq, dtype=np.float32)
    bk = np.asarray(bk, dtype=np.float32)
    bv = np.asarray(bv, dtype=np.float32)
    bo = np.asarray(bo, dtype=np.float32)

    nc = _get_nc()
    cosT, sinS = _rope_tables()

    def qk_pack(w, g):
        ws = w[g * 1024 : (g + 1) * 1024, :]
        return np.ascontiguousarray(
            ws.reshape(HL, 128, KO, 128).transpose(0, 2, 3, 1).astype(NPBF)
        )

    packs = []
    for g in range(2):
        wv_s = wv[g * 1024 : (g + 1) * 1024, :]
        wv_p = np.ascontiguousarray(
            wv_s.reshape(HL * DH, KO, 128).transpose(1, 2, 0).astype(NPBF)
        )
        wo_s = wo[:, g * 1024 : (g + 1) * 1024]
        wo_p = np.ascontiguousarray(
            wo_s.reshape(D, HL, 128).transpose(1, 2, 0).astype(NPBF)
        )
        bq_p = np.ascontiguousarray(
            (bq[g * 1024 : (g + 1) * 1024] * QSCALE).reshape(HL, 128).T
        )
        bk_p = np.ascontiguousarray(bk[g * 1024 : (g + 1) * 1024].reshape(HL, 128).T)
        packs.append(
            dict(
                wq=qk_pack(wq, g),
                wk=qk_pack(wk, g),
                wv=wv_p,
                wo=wo_p,
                bq=bq_p,
                bk=bk_p,
            )
        )

    in_maps = []
    xts = [
        np.ascontiguousarray(x[b].T.astype(NPBF)).reshape(KO, 128, S)
        for b in range(B)
    ]
    for c in range(8):
        b, g = c // 2, c % 2
        m = dict(packs[g])
        m["xt"] = xts[b]
        m["cosT"] = cosT
        m["sinS"] = sinS
        in_maps.append(m)

    res = run_bass_kernel_spmd(
        nc,
        in_maps,
        core_ids=list(range(8)),
        trace=_trace,
        tmpdir=_tmpdir,
    )

    bo_eff = bo + wo @ bv
    out = np.empty((B, S, D), dtype=np.float32)
    for b in range(B):
        acc = np.zeros((S, D), dtype=np.float32)
        for c in (2 * b, 2 * b + 1):
            for p in range(len(P3_PARTS)):
                acc += res.results[c][f"out{p}"].astype(np.float32)
        out[b] = acc + bo_eff[None, :]
    if _trace:
        kernel.last_result = res
    return out


# revision 35
# speedup vs baseline: 1.1175x; 1.0111x over previous
"""Multi-head attention (RoPE, non-causal) on 8 Trainium2 NeuronCores.

Problem: x[4,2048,2048] fp32; wq/wk/wv/wo [2048,2048]; biases [2048].
  q,k,v = x@w.T+b per 16 heads of dim 128; rope(q,k); softmax(q k^T/sqrt(128));
  out = (attn@v)@wo.T + bo.

Sharding: core c = 2*b + g -> batch b, head-group g (8 heads each).
Each core computes a partial output (its 8 heads) for its batch over the full
sequence; the host sums the partials (the wo contraction splits cleanly over
head groups) and adds bo_eff = bo + wo@bv (the V bias folds out exactly
because softmax rows sum to 1).

v3 (tail + engine-balance rework over v2's software pipeline):
  * P3 partials regrouped to [(0,4),(4,7),(7,8)]: partial 0 (heads 0-3,
    DRAM-bounced ct) fills heads 4-6; partial 1 (heads 4-6, normalized ct
    kept in SBUF - no spill/reload) fills head 7; partial 2 (head 7) runs
    in the tail, its matmuls emitted after the leftover partial-1 pieces so
    the PE stays busy under head 7's softmax-denominator DMA chain.
  * P3 PSUM->SBUF copies alternate between the Vector and Scalar engines
    (v2 put all of them on Vector, which serialized the tail at CAST speed).
  * Startup: wv streams on the Scalar DMA queue in parallel with xt on the
    Sync queue, and a burst of dummy matmuls warms the PE HAM clock while
    the first 4MB lands, so the first real matmul runs at 2.4GHz ~14us in.
  * The unnormalized-ctx PSUM->SBUF copy is split in 4 so ctx PSUM banks
    free progressively for the next head's first ctx matmuls.
"""

import sys

if "/opt/trn_rl_repo" not in sys.path:
    sys.path.insert(0, "/opt/trn_rl_repo")

import ml_dtypes
import numpy as np

import concourse.bass as bass
import concourse.tile as tile
from concourse import bacc, mybir
from concourse.bass_utils import run_bass_kernel_spmd

F32 = mybir.dt.float32
BF16 = mybir.dt.bfloat16
NPBF = ml_dtypes.bfloat16

B, S, D = 4, 2048, 2048
H = 16
DH = 128
HL = 8  # heads per core
KO = D // 128  # 16 k-chunks
TB = S // 128  # 16 t-chunks
ROPE_THETA = 10000.0
QSCALE = 1.0 / np.sqrt(DH)

# P3 partial-output head ranges:
#   partial 0: heads 0-3 (ct bounced via DRAM), filler during heads 4-6
#   partial 1: heads 4-6 (ct in SBUF), filler during head 7 + tail overlap
#   partial 2: head 7 (ct in SBUF), tail
P3_PARTS = [(0, 4), (4, 7), (7, 8)]

_NC_CACHE = {}


def build_nc():
    nc = bacc.Bacc()

    xt_d = nc.declare_dram_parameter("xt", [KO, 128, S], BF16, isOutput=False)
    wq_d = nc.declare_dram_parameter("wq", [HL, KO, 128, 128], BF16, isOutput=False)
    wk_d = nc.declare_dram_parameter("wk", [HL, KO, 128, 128], BF16, isOutput=False)
    wv_d = nc.declare_dram_parameter("wv", [KO, 128, HL * DH], BF16, isOutput=False)
    wo_d = nc.declare_dram_parameter("wo", [HL, 128, D], BF16, isOutput=False)
    cos_d = nc.declare_dram_parameter("cosT", [128, S], BF16, isOutput=False)
    sin_d = nc.declare_dram_parameter("sinS", [128, S], BF16, isOutput=False)
    bq_d = nc.declare_dram_parameter("bq", [128, HL], F32, isOutput=False)
    bk_d = nc.declare_dram_parameter("bk", [128, HL], F32, isOutput=False)
    out_ds = [
        nc.declare_dram_parameter(f"out{p}", [S, D], BF16, isOutput=True)
        for p in range(len(P3_PARTS))
    ]

    v_d = nc.dram_tensor("v_spill", [TB, 128, HL * DH], BF16)
    ct_d = nc.dram_tensor("ct_spill", [4, 128, S], BF16)  # heads 0-3 only
    den_d = nc.dram_tensor("den_bounce", [HL, 1, S], F32)
    rden_d = nc.dram_tensor("rden_bounce", [HL, 1, S], BF16)

    with tile.TileContext(nc) as tc:
        with (
            tc.tile_pool(name="xt_pool", bufs=1) as xt_pool,
            tc.tile_pool(name="const_pool", bufs=1) as const_pool,
            tc.tile_pool(name="w_pool", bufs=4) as w_pool,
            tc.tile_pool(name="qs_pool", bufs=2) as qs_pool,
            tc.tile_pool(name="qb_pool", bufs=4) as qb_pool,
            tc.tile_pool(name="v2_pool", bufs=2) as v2_pool,
            tc.tile_pool(name="et_pool", bufs=2) as et_pool,
            tc.tile_pool(name="pden_pool", bufs=2) as pden_pool,
            tc.tile_pool(name="den_pool", bufs=2) as den_pool,
            tc.tile_pool(name="norm_pool", bufs=2) as norm_pool,
            tc.tile_pool(name="psum", bufs=1, space="PSUM") as psum,
        ):
            # ---------------- prologue DMAs ----------------
            xt_sb = xt_pool.tile([128, KO, S], BF16)

            cos_sb = const_pool.tile([128, S], BF16)
            sin_sb = const_pool.tile([128, S], BF16)
            bq_sb = const_pool.tile([128, HL], F32)
            bk_sb = const_pool.tile([128, HL], F32)
            ones_sb = const_pool.tile([128, 1], BF16)
            warm_sb = const_pool.tile([128, 512], BF16)
            nc.vector.memset(ones_sb[:], 1.0)
            nc.vector.memset(warm_sb[:], 0.0)

            # PE warm-up: dummy matmuls with no data dependencies run while
            # the first input DMAs land (~21us for xt_q0+wv0 on two queues),
            # flipping the HAM clock gate to 8/8 before the first real
            # matmul issues and keeping it there.
            for _ in range(44):
                wps = psum.tile([128, 512], F32, tag="sc", bufs=2, name="warm_ps")
                nc.tensor.matmul(
                    wps[:], warm_sb[:, 0:128], warm_sb[:], start=True, stop=True
                )

            # per-head weight tiles (ring of 4: two heads in flight)
            w_tiles = {}

            def dma_w(h, queue=None):
                q = queue if queue is not None else nc.sync
                for nm, w_d in (("q", wq_d), ("k", wk_d)):
                    t = w_pool.tile([128, KO, 128], BF16, tag="w", bufs=4, name=f"w_{nm}")
                    q.dma_start(
                        out=t[:], in_=w_d[h].rearrange("k p m -> p k m")
                    )
                    w_tiles[(h, nm)] = t

            v2_tiles = {}

            def dma_v2(h):
                t = v2_pool.tile([128, TB, DH], BF16, name="v2")
                nc.sync.dma_start(
                    out=t[:],
                    in_=v_d[:, :, h * DH : (h + 1) * DH].rearrange("t p m -> p t m"),
                )
                v2_tiles[h] = t

            # qb tiles (rope'd q^T / k^T, bf16, ring of 4)
            qb_tiles = {}
            pden_tiles = {}
            cux_tiles = {}

            # ---------- filler piece machinery ----------
            def proj_pieces(h):
                """Q/K projection + rope for head h, as a list of closures."""
                pieces = []
                state = {}

                def chunk_first(nm, n, h=h):
                    def run():
                        w_sb = w_tiles[(h, nm)]
                        ps = psum.tile([128, 512], F32, tag="proj", bufs=2, name="proj_ps")
                        state[(nm, n)] = ps
                        for ko in range(8):
                            nc.tensor.matmul(
                                ps[:],
                                w_sb[:, ko, :],
                                xt_sb[:, ko, n * 512 : (n + 1) * 512],
                                start=(ko == 0),
                                stop=False,
                            )

                    return run

                def chunk_second(nm, n, h=h):
                    def run():
                        w_sb = w_tiles[(h, nm)]
                        ps = state[(nm, n)]
                        for ko in range(8, KO):
                            nc.tensor.matmul(
                                ps[:],
                                w_sb[:, ko, :],
                                xt_sb[:, ko, n * 512 : (n + 1) * 512],
                                start=False,
                                stop=(ko == KO - 1),
                            )
                        b_sb, scale = (
                            (bq_sb, QSCALE) if nm == "q" else (bk_sb, 1.0)
                        )
                        qs = state[("qs", nm)]
                        nc.vector.tensor_scalar(
                            out=qs[:, n * 512 : (n + 1) * 512],
                            in0=ps[:],
                            scalar1=scale,
                            scalar2=b_sb[:, h : h + 1],
                            op0=mybir.AluOpType.mult,
                            op1=mybir.AluOpType.add,
                        )

                    return run

                def rot_dma(nm):
                    # shuffle the halves straight into the qb tile (no
                    # separate rot buffer); rope then runs in place
                    def run():
                        qs = state[("qs", nm)]
                        qb = qb_pool.tile([128, S], BF16, tag="qb", bufs=4, name=f"qb_{nm}")
                        state[("qb", nm)] = qb
                        nc.sync.dma_start(out=qb[0:64, :], in_=qs[64:128, :])
                        nc.sync.dma_start(out=qb[64:128, :], in_=qs[0:64, :])

                    return run

                def rope(nm, h=h):
                    def run():
                        qs = state[("qs", nm)]
                        qb = state[("qb", nm)]
                        nc.vector.tensor_mul(out=qb[:], in0=qb[:], in1=sin_sb[:])
                        nc.vector.tensor_mul(out=qs[:], in0=qs[:], in1=cos_sb[:])
                        nc.vector.tensor_add(out=qb[:], in0=qb[:], in1=qs[:])
                        qb_tiles[(h, nm)] = qb

                    return run

                for nm in ("q", "k"):

                    def alloc_qs(nm=nm):
                        state[("qs", nm)] = qs_pool.tile([128, S], BF16, tag="qs", bufs=2, name=f"qs_{nm}")

                    pieces.append(alloc_qs)
                    for n in range(4):
                        pieces.append(chunk_first(nm, n))
                        pieces.append(chunk_second(nm, n))
                    pieces.append(rot_dma(nm))
                    pieces.append(rope(nm))
                return pieces

            def den_mm(h):
                """Column sums of pden: 4 matmuls into [1,512] psum rows,
                vector copies to SBUF, DMA to the den bounce buffer."""
                pden = pden_tiles[h]
                for n in range(4):
                    dps = psum.tile([1, 512], F32, tag="proj", bufs=2, name="dps")
                    nc.tensor.matmul(
                        dps[:],
                        ones_sb[:],
                        pden[:, n * 512 : (n + 1) * 512],
                        start=True,
                        stop=True,
                    )
                    dsb = den_pool.tile([1, 512], F32, tag="den", bufs=2, name="dsb")
                    nc.vector.tensor_copy(out=dsb[:], in_=dps[:])
                    nc.sync.dma_start(
                        out=den_d[h][:, n * 512 : (n + 1) * 512], in_=dsb[:]
                    )

            def den_pieces(h):
                """Softmax denominator readback + ct normalize (+ spill for
                h<4).  The den_mm part is emitted separately in the head's
                epilogue."""
                pieces = []
                state = {}

                def den_out():
                    # transposed read-back: [128, 16] so reciprocal is cheap
                    d128 = den_pool.tile([128, 16], F32, tag="d128", bufs=2, name="d128")
                    state["d128"] = d128
                    den_ap = den_d[h]
                    src = bass.AP(
                        tensor=den_ap.tensor,
                        offset=den_ap.offset,
                        ap=[[16, 128], [1, 16]],
                    )
                    nc.sync.dma_start(out=d128[:], in_=src)

                def recip():
                    d128 = state["d128"]
                    nc.vector.reciprocal(out=d128[:], in_=d128[:])
                    d128b = den_pool.tile([128, 16], BF16, tag="d128b", bufs=2, name="d128b")
                    nc.vector.tensor_copy(out=d128b[:], in_=d128[:])
                    rden_ap = rden_d[h]
                    dst = bass.AP(
                        tensor=rden_ap.tensor,
                        offset=rden_ap.offset,
                        ap=[[16, 128], [1, 16]],
                    )
                    nc.sync.dma_start(out=dst, in_=d128b[:])
                    # broadcast back across partitions
                    bc = norm_pool.tile([128, S], BF16, tag="bc", bufs=1, name="bc")
                    state["bc"] = bc
                    bsrc = bass.AP(
                        tensor=rden_ap.tensor,
                        offset=rden_ap.offset,
                        ap=[[0, 128]] + list(rden_ap.ap[1:]),
                    )
                    nc.sync.dma_start(out=bc[:], in_=bsrc)

                def norm():
                    cu = state["cu"]
                    bc = state["bc"]
                    if h < 4:
                        # normalize in place, then spill
                        nc.vector.tensor_mul(out=cu[:], in0=cu[:], in1=bc[:])
                        nc.sync.dma_start(out=ct_d[h], in_=cu[:])
                    else:
                        cux = cux_pool.tile([128, S], BF16, name="cux")
                        nc.vector.tensor_mul(out=cux[:], in0=cu[:], in1=bc[:])
                        cux_tiles[h] = cux

                pieces.append(lambda: den_mm(h))
                pieces.append(den_out)
                pieces.append(recip)
                pieces.append(norm)
                return pieces, state

            def p3_copy(ps, osb, n):
                # alternate copy engine: vector for even n, scalar for odd
                sl = slice(n * 512, (n + 1) * 512)
                if n % 2 == 0:
                    nc.vector.tensor_copy(out=osb[:, sl], in_=ps[:])
                else:
                    nc.scalar.copy(out=osb[:, sl], in_=ps[:])

            # In the tail the sc psum tag is idle, so P3 groups alternate
            # sc/proj tags there: 4 banks in flight instead of 2, which
            # decouples the MM -> copy -> MM serialization.
            p3_alt = {"on": False, "i": 0}

            def p3_pieces(part):
                """Output-projection groups for P3 partial `part`."""
                c0, c1 = P3_PARTS[part]
                ncs = c1 - c0
                use_sbuf = c0 >= 4
                pieces = []
                state = {}

                def cts_dma(m):
                    def run():
                        t = cts_pool.tile([128, ncs, 128], BF16, name="cts")
                        state[("cts", m)] = t
                        nc.sync.dma_start(
                            out=t[:],
                            in_=ct_d[c0:c1, :, m * 128 : (m + 1) * 128].rearrange(
                                "c p m2 -> p c m2"
                            ),
                        )

                    return run

                def alloc_osb(m):
                    def run():
                        state[("osb", m)] = osb_pool.tile([128, D], BF16, name="osb")

                    return run

                def group(m, n):
                    def run():
                        if p3_alt["on"]:
                            p3_alt["i"] += 1
                            tag = "sc" if p3_alt["i"] % 2 else "proj"
                        else:
                            tag = "proj"
                        ps = psum.tile([128, 512], F32, tag=tag, bufs=2, name="p3_ps")
                        for i in range(ncs):
                            if use_sbuf:
                                lhsT = cux_tiles[c0 + i][:, m * 128 : (m + 1) * 128]
                            else:
                                lhsT = state[("cts", m)][:, i, :]
                            nc.tensor.matmul(
                                ps[:],
                                lhsT,
                                wo_sb[:, c0 + i, n * 512 : (n + 1) * 512],
                                start=(i == 0),
                                stop=(i == ncs - 1),
                            )
                        osb = state[("osb", m)]
                        p3_copy(ps, osb, n)
                        if n == 3:
                            nc.sync.dma_start(
                                out=out_ds[part][m * 128 : (m + 1) * 128, :],
                                in_=osb[:],
                            )

                    return run

                if not use_sbuf:
                    pieces.append(cts_dma(0))
                for m in range(TB):
                    pieces.append(alloc_osb(m))
                    if not use_sbuf and m + 1 < TB:
                        pieces.append(cts_dma(m + 1))
                    for n in range(4):
                        pieces.append(group(m, n))
                return pieces

            # ---------------- V phase (with head-0 projection as filler) ---
            with tc.tile_pool(name="wv_pool", bufs=1) as wv_pool, tc.tile_pool(
                name="vout_pool", bufs=3
            ) as vout_pool:
                wv_sb = wv_pool.tile([128, KO, HL * DH], BF16)

                def xt_dma(q):
                    # column halves: 2KB contiguous runs per (ko, p) descriptor
                    # (quarters only got 1KB and ran ~25% slower)
                    sl = slice(q * 1024, (q + 1) * 1024)
                    nc.sync.dma_start(
                        out=xt_sb[:, :, sl],
                        in_=xt_d[:, :, sl].rearrange("k p s -> p k s"),
                    )

                def wv_dma(half):
                    # scalar DMA queue: streams in parallel with xt on sync
                    sl = slice(half * 512, (half + 1) * 512)
                    nc.scalar.dma_start(
                        out=wv_sb[:, :, sl],
                        in_=wv_d[:, :, sl].rearrange("k p m -> p k m"),
                    )

                # The sync DMA queue (~190GB/s) carries ONLY the xt quarters
                # so V units are never starved (tb4/8/12 need q1/q2/q3 at
                # ~40/57/74us; they land ~30/40/51us).  Everything else for
                # the V phase + head 0/1 rides the scalar queue: wv half 0
                # (first V matmul), head-0 q/k weights (proj filler ~27us),
                # rope/bias constants (~31us), head-1 weights, wv half 1
                # (V unit 16, ~91us).
                xt_dma(0)
                wv_dma(0)
                dma_w(0, queue=nc.scalar)
                nc.scalar.dma_start(out=cos_sb[:], in_=cos_d[:])
                nc.scalar.dma_start(out=sin_sb[:], in_=sin_d[:])
                nc.scalar.dma_start(out=bq_sb[:], in_=bq_d[:])
                nc.scalar.dma_start(out=bk_sb[:], in_=bk_d[:])
                dma_w(1, queue=nc.scalar)
                wv_dma(1)
                xt_dma(1)

                vfill = proj_pieces(0)
                vi = 0
                for nf in range(2):
                    for tb in range(TB):
                        vps = psum.tile([128, 512], F32, tag="sc", bufs=2)
                        for ko in range(KO):
                            nc.tensor.matmul(
                                vps[:],
                                xt_sb[:, ko, tb * 128 : (tb + 1) * 128],
                                wv_sb[:, ko, nf * 512 : (nf + 1) * 512],
                                start=(ko == 0),
                                stop=(ko == KO - 1),
                            )
                        vsb = vout_pool.tile([128, 512], BF16)
                        nc.vector.tensor_copy(out=vsb[:], in_=vps[:])
                        nc.sync.dma_start(
                            out=v_d[tb, :, nf * 512 : (nf + 1) * 512], in_=vsb[:]
                        )
                        # ~20 filler pieces over 32 V units
                        unit = nf * TB + tb
                        want = ((unit + 1) * len(vfill)) // 32
                        while vi < want:
                            vfill[vi]()
                            vi += 1
                        if nf == 0 and tb == TB - 1:
                            dma_v2(0)
                while vi < len(vfill):
                    vfill[vi]()
                    vi += 1

            # ---------------- head loop ----------------
            with (
                tc.tile_pool(name="wo_pool", bufs=1) as wo_pool,
                tc.tile_pool(name="cts_pool", bufs=2) as cts_pool,
                tc.tile_pool(name="osb_pool", bufs=2) as osb_pool,
                tc.tile_pool(name="cux_pool", bufs=3) as cux_pool,
            ):
              wo_sb = wo_pool.tile([128, HL, D], BF16)
              for n in range(4):
                sl = slice(n * 512, (n + 1) * 512)
                nc.scalar.dma_start(
                    out=wo_sb[:, :, sl],
                    in_=wo_d[:, :, sl].rearrange("c p m -> p c m"),
                )
              den_states = {}
              cu7_tile = []
              p3_p0 = p3_pieces(0)
              p0_cuts = [0, len(p3_p0) // 3, 2 * len(p3_p0) // 3, len(p3_p0)]
              p3_p1 = None
              p1_li = 0
              for h in range(HL):
                  # filler: den(h-1) pieces interleave with the next head's
                  # projection chunks (so early slots stay matmul-dense
                  # instead of exp-bound), the whole list paced over 14 of
                  # the 16 slots; late pieces (P3 groups) spread over slots
                  # 4..15.
                  den_p = []
                  rest = []
                  fill_late = []
                  if h >= 1:
                      dp, _st = den_states[h - 1]
                      den_p = list(dp)
                  if h + 1 < HL:
                      rest.extend(proj_pieces(h + 1))
                      rest.insert(8, lambda hh=h + 1: dma_v2(hh))
                      if h + 2 < HL:
                          rest.insert(6, lambda hh=h + 2: dma_w(hh))
                  fill = []
                  for piece in den_p:
                      fill.append(piece)
                      if rest:
                          fill.append(rest.pop(0))
                  fill.extend(rest)
                  if h in (4, 5, 6):
                      # spread partial-0 groups over heads 4-6
                      fill_late.extend(p3_p0[p0_cuts[h - 4] : p0_cuts[h - 3]])
                  if h == 7:
                      # partial 1 (heads 4-6): ~12/16 fills head 7; the
                      # remainder overlaps the tail's den(7) tiny matmuls
                      p3_p1 = p3_pieces(1)
                      fill_late.extend(p3_p1[: (len(p3_p1) * 12) // 16])

                  qt = qb_tiles[(h, "q")]
                  kt = qb_tiles[(h, "k")]
                  v2_sb = v2_tiles[h]

                  ctx_ps = psum.tile([128, S], F32, tag="ctx", bufs=1)
                  pden = pden_pool.tile([128, S], BF16)
                  pden_tiles[h] = pden

                  fi = 0
                  li = 0
                  for tb in range(TB):
                      et = et_pool.tile([128, S], BF16)
                      for n in range(4):
                          sc = psum.tile([128, 512], F32, tag="sc", bufs=2)
                          nc.tensor.matmul(
                              sc[:],
                              kt[:, tb * 128 : (tb + 1) * 128],
                              qt[:, n * 512 : (n + 1) * 512],
                              start=True,
                              stop=True,
                          )
                          nc.scalar.activation(
                              out=et[:, n * 512 : (n + 1) * 512],
                              in_=sc[:],
                              func=mybir.ActivationFunctionType.Exp,
                          )
                      # filler between scores and ctx.  Head 7's fill is just
                      # den(6), and partial 1 (which reads cux6 produced by
                      # its norm piece) starts at slot 4 -- so ramp it in 3
                      # slots there instead of 14.
                      ramp = 3 if h == 7 else 14
                      want = min(len(fill), ((tb + 1) * len(fill) + ramp - 1) // ramp)
                      while fi < want:
                          fill[fi]()
                          fi += 1
                      # fill_late starts at slot 4: the first P3 group of a
                      # partial reads ct written by den pieces that are
                      # emitted in this head's slots 0-2
                      if tb >= 4:
                          want_l = ((tb - 3) * len(fill_late)) // (TB - 4)
                          while li < want_l:
                              fill_late[li]()
                              li += 1
                      for n in range(4):
                          nc.tensor.matmul(
                              ctx_ps[:, n * 512 : (n + 1) * 512],
                              v2_sb[:, tb, :],
                              et[:, n * 512 : (n + 1) * 512],
                              start=(tb == 0),
                              stop=(tb == TB - 1),
                          )
                      if tb == 0:
                          nc.vector.tensor_copy(out=pden[:], in_=et[:])
                      else:
                          nc.vector.tensor_add(out=pden[:], in0=pden[:], in1=et[:])
                  if h == 7:
                      p1_li = li

                  # free ctx PSUM quickly: unnormalized ctx^T to SBUF (bf16),
                  # in 4 chunks so banks release progressively.  On VECTOR:
                  # the scalar queue must stay clear for the next head's
                  # slot-0 exps (in-order queue).
                  cu = norm_pool.tile([128, S], BF16, tag="cu", bufs=1)
                  for n in range(4):
                      sl = slice(n * 512, (n + 1) * 512)
                      nc.vector.tensor_copy(out=cu[:, sl], in_=ctx_ps[:, sl])
                  if h < 7:
                      dp, st = den_pieces(h)
                      den_states[h] = (dp, st)
                      st["cu"] = cu
                  else:
                      cu7_tile.append(cu)

              # ---------------- tail ----------------
              # Head 7's softmax denominator never leaves the chip: 16 tiny
              # matmuls (pden7 chunk^T @ ones) write den columns into ONE
              # psum bank as [128, 16], reciprocal'd straight into SBUF.
              # Partial 2's matmuls then run on the UNNORMALIZED cu7 (no
              # dependency on the denominator at all) and the 1/den scaling
              # happens in the PSUM->SBUF copies via per-partition scalars.
              # Leftover partial-1 pieces cover the tiny-matmul latency.
              # P3 psum tiles alternate sc/proj tags here (sc is free).
              p3_alt["on"] = True
              cu7 = cu7_tile[0]
              pden7 = pden_tiles[7]

              # p1 remainder first: its matmuls run immediately (no den
              # dependency) while pden7's last adds drain on vector
              while p1_li < len(p3_p1):
                  p3_p1[p1_li]()
                  p1_li += 1

              dcols = psum.tile([128, 16], F32, tag="sc", bufs=2, name="dcols")
              for m in range(TB):
                  # start=True only on the first: start clears has_written
                  # for the WHOLE bank (would orphan earlier columns)
                  nc.tensor.matmul(
                      dcols[:, m : m + 1],
                      pden7[:, m * 128 : (m + 1) * 128],
                      ones_sb[:],
                      start=(m == 0),
                      stop=(m == TB - 1),
                      skip_group_check=True,
                  )
              mcols = den_pool.tile([128, 16], F32, tag="mcols", bufs=1, name="mcols")
              nc.vector.reciprocal(out=mcols[:], in_=dcols[:])

              for m in range(TB):
                  osb = osb_pool.tile([128, D], BF16, name="osb")
                  for n in range(4):
                      p3_alt["i"] += 1
                      tag = "sc" if p3_alt["i"] % 2 else "proj"
                      ps = psum.tile([128, 512], F32, tag=tag, bufs=2, name="p2_ps")
                      nc.tensor.matmul(
                          ps[:],
                          cu7[:, m * 128 : (m + 1) * 128],
                          wo_sb[:, 7, n * 512 : (n + 1) * 512],
                          start=True,
                          stop=True,
                      )
                      sl = slice(n * 512, (n + 1) * 512)
                      if n % 2 == 0:
                          nc.vector.tensor_scalar_mul(
                              out=osb[:, sl], in0=ps[:], scalar1=mcols[:, m : m + 1]
                          )
                      else:
                          nc.scalar.activation(
                              out=osb[:, sl],
                              in_=ps[:],
                              func=mybir.ActivationFunctionType.Copy,
                              scale=mcols[:, m : m + 1],
                          )
                  nc.sync.dma_start(
                      out=out_ds[2][m * 128 : (m + 1) * 128, :], in_=osb[:]
                  )

    nc.finalize()
    return nc


def _get_nc():
    if "nc" not in _NC_CACHE:
        _NC_CACHE["nc"] = build_nc()
    return _NC_CACHE["nc"]


def _rope_tables():
    inv_freq = 1.0 / (ROPE_THETA ** (np.arange(0, DH, 2, dtype=np.float32) / DH))
    freqs = np.arange(S, dtype=np.float32)[:, None] * inv_freq[None, :]
    emb = np.concatenate([freqs, freqs], axis=-1)  # [S, 128]
    cosT = np.ascontiguousarray(np.cos(emb).T.astype(NPBF))  # [128, S]
    sinS = np.sin(emb).T.astype(np.float32).copy()
    sinS[0:64, :] *= -1.0  # sign-folded rotate_half
    return cosT, np.ascontiguousarray(sinS.astype(NPBF))


def kernel(x, wq, bq, wk, bk, wv, bv, wo, bo, _trace=False, _tmpdir=None):
    x = np.asarray(x, dtype=np.float32)
    wq = np.asarray(wq, dtype=np.float32)
    wk = np.asarray(wk, dtype=np.float32)
    wv = np.asarray(wv, dtype=np.float32)
    wo = np.asarray(wo, dtype=np.float32)
    bq = np.asarray(bq, dtype=np.float32)
    bk = np.asarray(bk, dtype=np.float32)
    bv = np.asarray(bv, dtype=np.float32)
    bo = np.asarray(bo, dtype=np.float32)

    nc = _get_nc()
    cosT, sinS = _rope_tables()

    def qk_pack(w, g):
        ws = w[g * 1024 : (g + 1) * 1024, :]
        return np.ascontiguousarray(
            ws.reshape(HL, 128, KO, 128).transpose(0, 2, 3, 1).astype(NPBF)
        )

    packs = []
    for g in range(2):
        wv_s = wv[g * 1024 : (g + 1) * 1024, :]
        wv_p = np.ascontiguousarray(
            wv_s.reshape(HL * DH, KO, 128).transpose(1, 2, 0).astype(NPBF)
        )
        wo_s = wo[:, g * 1024 : (g + 1) * 1024]
        wo_p = np.ascontiguousarray(
            wo_s.reshape(D, HL, 128).transpose(1, 2, 0).astype(NPBF)
        )
        bq_p = np.ascontiguousarray(
            (bq[g * 1024 : (g + 1) * 1024] * QSCALE).reshape(HL, 128).T
        )
        bk_p = np.ascontiguousarray(bk[g * 1024 : (g + 1) * 1024].reshape(HL, 128).T)
        packs.append(
            dict(
                wq=qk_pack(wq, g),
                wk=qk_pack(wk, g),
                wv=wv_p,
                wo=wo_p,
                bq=bq_p,
                bk=bk_p,
            )
        )

    in_maps = []
    xts = [
        np.ascontiguousarray(x[b].T.astype(NPBF)).reshape(KO, 128, S)
        for b in range(B)
    ]
    for c in range(8):
        b, g = c // 2, c % 2
        m = dict(packs[g])
        m["xt"] = xts[b]
        m["cosT"] = cosT
        m["sinS"] = sinS
        in_maps.append(m)

    res = run_bass_kernel_spmd(
        nc,
        in_maps,
        core_ids=list(range(8)),
        trace=_trace,
        tmpdir=_tmpdir,
    )

    bo_eff = bo + wo @ bv
    out = np.empty((B, S, D), dtype=np.float32)
    for b in range(B):
        acc = np.zeros((S, D), dtype=np.float32)
        for c in (2 * b, 2 * b + 1):
            for p in range(len(P3_PARTS)):
                acc += res.results[c][f"out{p}"].astype(np.float32)
        out[b] = acc + bo_eff[None, :]
    if _trace:
        kernel.last_result = res
    return out


# revision 42
# speedup vs baseline: 1.1240x; 1.0058x over previous
"""Multi-head attention (RoPE, non-causal) on 8 Trainium2 NeuronCores.

Problem: x[4,2048,2048] fp32; wq/wk/wv/wo [2048,2048]; biases [2048].
  q,k,v = x@w.T+b per 16 heads of dim 128; rope(q,k); softmax(q k^T/sqrt(128));
  out = (attn@v)@wo.T + bo.

Sharding: core c = 2*b + g -> batch b, head-group g (8 heads each).
Each core computes a partial output (its 8 heads) for its batch over the full
sequence; the host sums the partials (the wo contraction splits cleanly over
head groups) and adds bo_eff = bo + wo@bv (the V bias folds out exactly
because softmax rows sum to 1).

v3 (tail + engine-balance rework over v2's software pipeline):
  * P3 partials regrouped to [(0,4),(4,7),(7,8)]: partial 0 (heads 0-3,
    DRAM-bounced ct) fills heads 4-6; partial 1 (heads 4-6, normalized ct
    kept in SBUF - no spill/reload) fills head 7; partial 2 (head 7) runs
    in the tail, its matmuls emitted after the leftover partial-1 pieces so
    the PE stays busy under head 7's softmax-denominator DMA chain.
  * P3 PSUM->SBUF copies alternate between the Vector and Scalar engines
    (v2 put all of them on Vector, which serialized the tail at CAST speed).
  * Startup: wv streams on the Scalar DMA queue in parallel with xt on the
    Sync queue, and a burst of dummy matmuls warms the PE HAM clock while
    the first 4MB lands, so the first real matmul runs at 2.4GHz ~14us in.
  * The unnormalized-ctx PSUM->SBUF copy is split in 4 so ctx PSUM banks
    free progressively for the next head's first ctx matmuls.
"""

import sys

if "/opt/trn_rl_repo" not in sys.path:
    sys.path.insert(0, "/opt/trn_rl_repo")

import ml_dtypes
import numpy as np

import concourse.bass as bass
import concourse.tile as tile
from concourse import bacc, mybir
from concourse.bass_utils import run_bass_kernel_spmd

F32 = mybir.dt.float32
BF16 = mybir.dt.bfloat16
NPBF = ml_dtypes.bfloat16

B, S, D = 4, 2048, 2048
H = 16
DH = 128
HL = 8  # heads per core
KO = D // 128  # 16 k-chunks
TB = S // 128  # 16 t-chunks
ROPE_THETA = 10000.0
QSCALE = 1.0 / np.sqrt(DH)

# P3 partial-output head ranges:
#   partial 0: heads 0-3 (ct bounced via DRAM), filler during heads 4-6
#   partial 1: heads 4-6 (ct in SBUF), filler during head 7 + tail overlap
#   partial 2: head 7 (ct in SBUF), tail
P3_PARTS = [(0, 4), (4, 7), (7, 8)]

_NC_CACHE = {}


def build_nc():
    nc = bacc.Bacc()

    xt_d = nc.declare_dram_parameter("xt", [KO, 128, S], BF16, isOutput=False)
    wq_d = nc.declare_dram_parameter("wq", [HL, KO, 128, 128], BF16, isOutput=False)
    wk_d = nc.declare_dram_parameter("wk", [HL, KO, 128, 128], BF16, isOutput=False)
    wv_d = nc.declare_dram_parameter("wv", [KO, 128, HL * DH], BF16, isOutput=False)
    wo_d = nc.declare_dram_parameter("wo", [HL, 128, D], BF16, isOutput=False)
    cos_d = nc.declare_dram_parameter("cosT", [128, S], BF16, isOutput=False)
    sin_d = nc.declare_dram_parameter("sinS", [128, S], BF16, isOutput=False)
    bq_d = nc.declare_dram_parameter("bq", [128, HL], F32, isOutput=False)
    bk_d = nc.declare_dram_parameter("bk", [128, HL], F32, isOutput=False)
    out_ds = [
        nc.declare_dram_parameter(f"out{p}", [S, D], BF16, isOutput=True)
        for p in range(len(P3_PARTS))
    ]

    v_d = nc.dram_tensor("v_spill", [TB, 128, HL * DH], BF16)
    ct_d = nc.dram_tensor("ct_spill", [4, 128, S], BF16)  # heads 0-3 only
    den_d = nc.dram_tensor("den_bounce", [HL, 1, S], F32)
    rden_d = nc.dram_tensor("rden_bounce", [HL, 1, S], BF16)

    with tile.TileContext(nc) as tc:
        with (
            tc.tile_pool(name="xt_pool", bufs=1) as xt_pool,
            tc.tile_pool(name="const_pool", bufs=1) as const_pool,
            tc.tile_pool(name="w_pool", bufs=4) as w_pool,
            tc.tile_pool(name="qs_pool", bufs=2) as qs_pool,
            tc.tile_pool(name="qb_pool", bufs=4) as qb_pool,
            tc.tile_pool(name="v2_pool", bufs=2) as v2_pool,
            tc.tile_pool(name="et_pool", bufs=2) as et_pool,
            tc.tile_pool(name="pden_pool", bufs=2) as pden_pool,
            tc.tile_pool(name="den_pool", bufs=2) as den_pool,
            tc.tile_pool(name="norm_pool", bufs=2) as norm_pool,
            tc.tile_pool(name="psum", bufs=1, space="PSUM") as psum,
        ):
            # ---------------- prologue DMAs ----------------
            xt_sb = xt_pool.tile([128, KO, S], BF16)

            cos_sb = const_pool.tile([128, S], BF16)
            sin_sb = const_pool.tile([128, S], BF16)
            bq_sb = const_pool.tile([128, HL], F32)
            bk_sb = const_pool.tile([128, HL], F32)
            ones_sb = const_pool.tile([128, 1], BF16)
            warm_sb = const_pool.tile([128, 512], BF16)
            nc.vector.memset(ones_sb[:], 1.0)
            nc.vector.memset(warm_sb[:], 0.0)

            # PE warm-up: dummy matmuls with no data dependencies run while
            # the first input DMAs land (~21us for xt_q0+wv0 on two queues),
            # flipping the HAM clock gate to 8/8 before the first real
            # matmul issues and keeping it there.
            for _ in range(44):
                wps = psum.tile([128, 512], F32, tag="sc", bufs=2, name="warm_ps")
                nc.tensor.matmul(
                    wps[:], warm_sb[:, 0:128], warm_sb[:], start=True, stop=True
                )

            # per-head weight tiles (ring of 4: two heads in flight)
            w_tiles = {}

            def dma_w(h, queue=None):
                q = queue if queue is not None else nc.sync
                for nm, w_d in (("q", wq_d), ("k", wk_d)):
                    t = w_pool.tile([128, KO, 128], BF16, tag="w", bufs=4, name=f"w_{nm}")
                    q.dma_start(
                        out=t[:], in_=w_d[h].rearrange("k p m -> p k m")
                    )
                    w_tiles[(h, nm)] = t

            v2_tiles = {}

            def dma_v2(h):
                t = v2_pool.tile([128, TB, DH], BF16, name="v2")
                nc.sync.dma_start(
                    out=t[:],
                    in_=v_d[:, :, h * DH : (h + 1) * DH].rearrange("t p m -> p t m"),
                )
                v2_tiles[h] = t

            # qb tiles (rope'd q^T / k^T, bf16, ring of 4)
            qb_tiles = {}
            pden_tiles = {}
            cux_tiles = {}

            # ---------- filler piece machinery ----------
            def proj_pieces(h):
                """Q/K projection + rope for head h, as a list of closures."""
                pieces = []
                state = {}

                def chunk_first(nm, n, h=h):
                    def run():
                        w_sb = w_tiles[(h, nm)]
                        ps = psum.tile([128, 512], F32, tag="proj", bufs=2, name="proj_ps")
                        state[(nm, n)] = ps
                        for ko in range(8):
                            nc.tensor.matmul(
                                ps[:],
                                w_sb[:, ko, :],
                                xt_sb[:, ko, n * 512 : (n + 1) * 512],
                                start=(ko == 0),
                                stop=False,
                            )

                    return run

                def chunk_second(nm, n, h=h):
                    def run():
                        w_sb = w_tiles[(h, nm)]
                        ps = state[(nm, n)]
                        for ko in range(8, KO):
                            nc.tensor.matmul(
                                ps[:],
                                w_sb[:, ko, :],
                                xt_sb[:, ko, n * 512 : (n + 1) * 512],
                                start=False,
                                stop=(ko == KO - 1),
                            )
                        b_sb, scale = (
                            (bq_sb, QSCALE) if nm == "q" else (bk_sb, 1.0)
                        )
                        qs = state[("qs", nm)]
                        nc.vector.tensor_scalar(
                            out=qs[:, n * 512 : (n + 1) * 512],
                            in0=ps[:],
                            scalar1=scale,
                            scalar2=b_sb[:, h : h + 1],
                            op0=mybir.AluOpType.mult,
                            op1=mybir.AluOpType.add,
                        )

                    return run

                def rot_dma(nm):
                    # shuffle the halves straight into the qb tile (no
                    # separate rot buffer); rope then runs in place
                    def run():
                        qs = state[("qs", nm)]
                        qb = qb_pool.tile([128, S], BF16, tag="qb", bufs=4, name=f"qb_{nm}")
                        state[("qb", nm)] = qb
                        nc.sync.dma_start(out=qb[0:64, :], in_=qs[64:128, :])
                        nc.sync.dma_start(out=qb[64:128, :], in_=qs[0:64, :])

                    return run

                def rope(nm, h=h):
                    def run():
                        qs = state[("qs", nm)]
                        qb = state[("qb", nm)]
                        nc.vector.tensor_mul(out=qb[:], in0=qb[:], in1=sin_sb[:])
                        nc.vector.tensor_mul(out=qs[:], in0=qs[:], in1=cos_sb[:])
                        nc.vector.tensor_add(out=qb[:], in0=qb[:], in1=qs[:])
                        qb_tiles[(h, nm)] = qb

                    return run

                for nm in ("q", "k"):

                    def alloc_qs(nm=nm):
                        state[("qs", nm)] = qs_pool.tile([128, S], BF16, tag="qs", bufs=2, name=f"qs_{nm}")

                    pieces.append(alloc_qs)
                    for n in range(4):
                        pieces.append(chunk_first(nm, n))
                        pieces.append(chunk_second(nm, n))
                    pieces.append(rot_dma(nm))
                    pieces.append(rope(nm))
                return pieces

            def den_mm_piece(h, n):
                """One column-sum chunk of pden: [1,512] psum row, vector
                copy to SBUF, DMA to the den bounce buffer.  Split in 4
                pieces so the shared proj psum ring never backs up behind
                the vector queue at a head boundary."""
                def run():
                    pden = pden_tiles[h]
                    dps = psum.tile([1, 512], F32, tag="proj", bufs=2, name="dps")
                    nc.tensor.matmul(
                        dps[:],
                        ones_sb[:],
                        pden[:, n * 512 : (n + 1) * 512],
                        start=True,
                        stop=True,
                    )
                    dsb = den_pool.tile([1, 512], F32, tag="den", bufs=2, name="dsb")
                    nc.vector.tensor_copy(out=dsb[:], in_=dps[:])
                    nc.sync.dma_start(
                        out=den_d[h][:, n * 512 : (n + 1) * 512], in_=dsb[:]
                    )

                return run

            def den_pieces(h):
                """Softmax denominator readback + ct normalize (+ spill for
                h<4).  The den_mm part is emitted separately in the head's
                epilogue."""
                pieces = []
                state = {}

                def den_out():
                    # transposed read-back: [128, 16] so reciprocal is cheap
                    d128 = den_pool.tile([128, 16], F32, tag="d128", bufs=2, name="d128")
                    state["d128"] = d128
                    den_ap = den_d[h]
                    src = bass.AP(
                        tensor=den_ap.tensor,
                        offset=den_ap.offset,
                        ap=[[16, 128], [1, 16]],
                    )
                    nc.sync.dma_start(out=d128[:], in_=src)

                def recip():
                    d128 = state["d128"]
                    nc.vector.reciprocal(out=d128[:], in_=d128[:])
                    d128b = den_pool.tile([128, 16], BF16, tag="d128b", bufs=2, name="d128b")
                    nc.vector.tensor_copy(out=d128b[:], in_=d128[:])
                    rden_ap = rden_d[h]
                    dst = bass.AP(
                        tensor=rden_ap.tensor,
                        offset=rden_ap.offset,
                        ap=[[16, 128], [1, 16]],
                    )
                    nc.sync.dma_start(out=dst, in_=d128b[:])
                    # broadcast back across partitions
                    bc = norm_pool.tile([128, S], BF16, tag="bc", bufs=1, name="bc")
                    state["bc"] = bc
                    bsrc = bass.AP(
                        tensor=rden_ap.tensor,
                        offset=rden_ap.offset,
                        ap=[[0, 128]] + list(rden_ap.ap[1:]),
                    )
                    nc.sync.dma_start(out=bc[:], in_=bsrc)

                def norm():
                    cu = state["cu"]
                    bc = state["bc"]
                    if h < 4:
                        # normalize in place, then spill
                        nc.vector.tensor_mul(out=cu[:], in0=cu[:], in1=bc[:])
                        nc.sync.dma_start(out=ct_d[h], in_=cu[:])
                    else:
                        cux = cux_pool.tile([128, S], BF16, name="cux")
                        nc.vector.tensor_mul(out=cux[:], in0=cu[:], in1=bc[:])
                        cux_tiles[h] = cux

                for n in range(4):
                    pieces.append(den_mm_piece(h, n))
                pieces.append(den_out)
                pieces.append(recip)
                pieces.append(norm)
                return pieces, state

            def p3_copy(ps, osb, n):
                # alternate copy engine: vector for even n, scalar for odd
                sl = slice(n * 512, (n + 1) * 512)
                if n % 2 == 0:
                    nc.vector.tensor_copy(out=osb[:, sl], in_=ps[:])
                else:
                    nc.scalar.copy(out=osb[:, sl], in_=ps[:])

            # In the tail the sc psum tag is idle, so P3 groups alternate
            # sc/proj tags there: 4 banks in flight instead of 2, which
            # decouples the MM -> copy -> MM serialization.
            p3_alt = {"on": False, "i": 0}

            def p3_pieces(part):
                """Output-projection groups for P3 partial `part`."""
                c0, c1 = P3_PARTS[part]
                ncs = c1 - c0
                use_sbuf = c0 >= 4
                pieces = []
                state = {}

                def cts_dma(m):
                    def run():
                        t = cts_pool.tile([128, ncs, 128], BF16, name="cts")
                        state[("cts", m)] = t
                        nc.sync.dma_start(
                            out=t[:],
                            in_=ct_d[c0:c1, :, m * 128 : (m + 1) * 128].rearrange(
                                "c p m2 -> p c m2"
                            ),
                        )

                    return run

                def alloc_osb(m):
                    def run():
                        state[("osb", m)] = osb_pool.tile([128, D], BF16, name="osb")

                    return run

                def group(m, n):
                    def run():
                        if p3_alt["on"]:
                            p3_alt["i"] += 1
                            tag = "sc" if p3_alt["i"] % 2 else "proj"
                        else:
                            tag = "proj"
                        ps = psum.tile([128, 512], F32, tag=tag, bufs=2, name="p3_ps")
                        for i in range(ncs):
                            if use_sbuf:
                                lhsT = cux_tiles[c0 + i][:, m * 128 : (m + 1) * 128]
                            else:
                                lhsT = state[("cts", m)][:, i, :]
                            nc.tensor.matmul(
                                ps[:],
                                lhsT,
                                wo_sb[:, c0 + i, n * 512 : (n + 1) * 512],
                                start=(i == 0),
                                stop=(i == ncs - 1),
                            )
                        osb = state[("osb", m)]
                        p3_copy(ps, osb, n)
                        if n == 3:
                            nc.sync.dma_start(
                                out=out_ds[part][m * 128 : (m + 1) * 128, :],
                                in_=osb[:],
                            )

                    return run

                if not use_sbuf:
                    pieces.append(cts_dma(0))
                    pieces.append(cts_dma(1))
                for m in range(TB):
                    pieces.append(alloc_osb(m))
                    if not use_sbuf and m + 2 < TB:
                        pieces.append(cts_dma(m + 2))
                    for n in range(4):
                        pieces.append(group(m, n))
                return pieces

            # ---------------- V phase (with head-0 projection as filler) ---
            with tc.tile_pool(name="wv_pool", bufs=1) as wv_pool, tc.tile_pool(
                name="vout_pool", bufs=3
            ) as vout_pool:
                wv_sb = wv_pool.tile([128, KO, HL * DH], BF16)

                def xt_dma(lo, hi):
                    # first two quarters land fast for V units 0-7, the back
                    # half rides 2KB descriptors for throughput
                    sl = slice(lo, hi)
                    nc.sync.dma_start(
                        out=xt_sb[:, :, sl],
                        in_=xt_d[:, :, sl].rearrange("k p s -> p k s"),
                    )

                def wv_dma(half):
                    # scalar DMA queue: streams in parallel with xt on sync
                    sl = slice(half * 512, (half + 1) * 512)
                    nc.scalar.dma_start(
                        out=wv_sb[:, :, sl],
                        in_=wv_d[:, :, sl].rearrange("k p m -> p k m"),
                    )

                # The sync DMA queue (~190GB/s) carries ONLY the xt quarters
                # so V units are never starved (tb4/8/12 need q1/q2/q3 at
                # ~40/57/74us; they land ~30/40/51us).  Everything else for
                # the V phase + head 0/1 rides the scalar queue: wv half 0
                # (first V matmul), head-0 q/k weights (proj filler ~27us),
                # rope/bias constants (~31us), head-1 weights, wv half 1
                # (V unit 16, ~91us).
                xt_dma(0, 512)
                wv_dma(0)
                dma_w(0, queue=nc.scalar)
                xt_dma(512, 1024)
                nc.scalar.dma_start(out=cos_sb[:], in_=cos_d[:])
                nc.scalar.dma_start(out=sin_sb[:], in_=sin_d[:])
                nc.scalar.dma_start(out=bq_sb[:], in_=bq_d[:])
                nc.scalar.dma_start(out=bk_sb[:], in_=bk_d[:])
                dma_w(1, queue=nc.scalar)
                wv_dma(1)
                xt_dma(1024, 2048)

                vfill = proj_pieces(0)
                vi = 0
                for nf in range(2):
                    for tb in range(TB):
                        vps = psum.tile([128, 512], F32, tag="sc", bufs=2)
                        for ko in range(KO):
                            nc.tensor.matmul(
                                vps[:],
                                xt_sb[:, ko, tb * 128 : (tb + 1) * 128],
                                wv_sb[:, ko, nf * 512 : (nf + 1) * 512],
                                start=(ko == 0),
                                stop=(ko == KO - 1),
                            )
                        vsb = vout_pool.tile([128, 512], BF16)
                        nc.vector.tensor_copy(out=vsb[:], in_=vps[:])
                        nc.sync.dma_start(
                            out=v_d[tb, :, nf * 512 : (nf + 1) * 512], in_=vsb[:]
                        )
                        # ~20 filler pieces over 32 V units
                        unit = nf * TB + tb
                        want = ((unit + 1) * len(vfill)) // 32
                        while vi < want:
                            vfill[vi]()
                            vi += 1
                        if nf == 0 and tb == TB - 1:
                            dma_v2(0)
                while vi < len(vfill):
                    vfill[vi]()
                    vi += 1

            # ---------------- head loop ----------------
            with (
                tc.tile_pool(name="wo_pool", bufs=1) as wo_pool,
                tc.tile_pool(name="cts_pool", bufs=3) as cts_pool,
                tc.tile_pool(name="osb_pool", bufs=2) as osb_pool,
                tc.tile_pool(name="cux_pool", bufs=3) as cux_pool,
            ):
              wo_sb = wo_pool.tile([128, HL, D], BF16)
              for n in range(4):
                sl = slice(n * 512, (n + 1) * 512)
                nc.scalar.dma_start(
                    out=wo_sb[:, :, sl],
                    in_=wo_d[:, :, sl].rearrange("c p m -> p c m"),
                )
              den_states = {}
              cu7_tile = []
              p3_p0 = p3_pieces(0)
              p0_cuts = [0, len(p3_p0) // 3, 2 * len(p3_p0) // 3, len(p3_p0)]
              p3_p1 = None
              p1_li = 0
              for h in range(HL):
                  # filler: den(h-1) pieces interleave with the next head's
                  # projection chunks (so early slots stay matmul-dense
                  # instead of exp-bound), the whole list paced over 14 of
                  # the 16 slots; late pieces (P3 groups) spread over slots
                  # 4..15.
                  den_p = []
                  rest = []
                  fill_late = []
                  if h >= 1:
                      dp, _st = den_states[h - 1]
                      den_p = list(dp)
                  if h + 1 < HL:
                      rest.extend(proj_pieces(h + 1))
                      rest.insert(8, lambda hh=h + 1: dma_v2(hh))
                      if h + 2 < HL:
                          rest.insert(6, lambda hh=h + 2: dma_w(hh))
                  fill = []
                  for piece in den_p:
                      fill.append(piece)
                      if rest:
                          fill.append(rest.pop(0))
                  fill.extend(rest)
                  if h in (4, 5, 6):
                      # spread partial-0 groups over heads 4-6
                      fill_late.extend(p3_p0[p0_cuts[h - 4] : p0_cuts[h - 3]])
                  if h == 7:
                      # partial 1 (heads 4-6): ~12/16 fills head 7; the
                      # remainder overlaps the tail's den(7) tiny matmuls
                      p3_p1 = p3_pieces(1)
                      fill_late.extend(p3_p1[: (len(p3_p1) * 12) // 16])

                  qt = qb_tiles[(h, "q")]
                  kt = qb_tiles[(h, "k")]
                  v2_sb = v2_tiles[h]

                  ctx_ps = psum.tile([128, S], F32, tag="ctx", bufs=1)
                  pden = pden_pool.tile([128, S], BF16)
                  pden_tiles[h] = pden

                  fi = 0
                  li = 0
                  for tb in range(TB):
                      et = et_pool.tile([128, S], BF16)
                      for n in range(4):
                          sc = psum.tile([128, 512], F32, tag="sc", bufs=2)
                          nc.tensor.matmul(
                              sc[:],
                              kt[:, tb * 128 : (tb + 1) * 128],
                              qt[:, n * 512 : (n + 1) * 512],
                              start=True,
                              stop=True,
                          )
                          nc.scalar.activation(
                              out=et[:, n * 512 : (n + 1) * 512],
                              in_=sc[:],
                              func=mybir.ActivationFunctionType.Exp,
                          )
                      # filler between scores and ctx.  Head 7's fill is just
                      # den(6), and partial 1 (which reads cux6 produced by
                      # its norm piece) starts at slot 4 -- so ramp it in 3
                      # slots there instead of 14.
                      ramp = 3 if h == 7 else 14
                      want = min(len(fill), ((tb + 1) * len(fill) + ramp - 1) // ramp)
                      while fi < want:
                          fill[fi]()
                          fi += 1
                      # fill_late starts once the ct its first group reads
                      # has been spilled: head 4 waits for norm(3) (emitted
                      # ~slot 5 now that den pieces weave wider), the rest
                      # start at slot 4.
                      ls = 6 if h == 4 else 4
                      if tb >= ls:
                          want_l = ((tb - ls + 1) * len(fill_late)) // (TB - ls)
                          while li < want_l:
                              fill_late[li]()
                              li += 1
                      for n in range(4):
                          nc.tensor.matmul(
                              ctx_ps[:, n * 512 : (n + 1) * 512],
                              v2_sb[:, tb, :],
                              et[:, n * 512 : (n + 1) * 512],
                              start=(tb == 0),
                              stop=(tb == TB - 1),
                          )
                      if tb == 0:
                          nc.vector.tensor_copy(out=pden[:], in_=et[:])
                      else:
                          nc.vector.tensor_add(out=pden[:], in0=pden[:], in1=et[:])
                  if h == 7:
                      p1_li = li

                  # free ctx PSUM quickly: unnormalized ctx^T to SBUF (bf16),
                  # in 4 chunks so banks release progressively.  On VECTOR:
                  # the scalar queue must stay clear for the next head's
                  # slot-0 exps (in-order queue).
                  cu = norm_pool.tile([128, S], BF16, tag="cu", bufs=1)
                  for n in range(4):
                      sl = slice(n * 512, (n + 1) * 512)
                      nc.vector.tensor_copy(out=cu[:, sl], in_=ctx_ps[:, sl])
                  if h < 7:
                      dp, st = den_pieces(h)
                      den_states[h] = (dp, st)
                      st["cu"] = cu
                  else:
                      cu7_tile.append(cu)

              # ---------------- tail ----------------
              # Head 7's softmax denominator never leaves the chip: 16 tiny
              # matmuls (pden7 chunk^T @ ones) write den columns into ONE
              # psum bank as [128, 16], reciprocal'd straight into SBUF.
              # Partial 2's matmuls then run on the UNNORMALIZED cu7 (no
              # dependency on the denominator at all) and the 1/den scaling
              # happens in the PSUM->SBUF copies via per-partition scalars.
              # Leftover partial-1 pieces cover the tiny-matmul latency.
              # P3 psum tiles alternate sc/proj tags here (sc is free).
              p3_alt["on"] = True
              cu7 = cu7_tile[0]
              pden7 = pden_tiles[7]

              # p1 remainder first: its matmuls run immediately (no den
              # dependency) while pden7's last adds drain on vector
              while p1_li < len(p3_p1):
                  p3_p1[p1_li]()
                  p1_li += 1

              dcols = psum.tile([128, 16], F32, tag="sc", bufs=2, name="dcols")
              for m in range(TB):
                  # start=True only on the first: start clears has_written
                  # for the WHOLE bank (would orphan earlier columns)
                  nc.tensor.matmul(
                      dcols[:, m : m + 1],
                      pden7[:, m * 128 : (m + 1) * 128],
                      ones_sb[:],
                      start=(m == 0),
                      stop=(m == TB - 1),
                      skip_group_check=True,
                  )
              mcols = den_pool.tile([128, 16], F32, tag="mcols", bufs=1, name="mcols")
              nc.vector.reciprocal(out=mcols[:], in_=dcols[:])

              for m in range(TB):
                  osb = osb_pool.tile([128, D], BF16, name="osb")
                  for n in range(4):
                      p3_alt["i"] += 1
                      tag = "sc" if p3_alt["i"] % 2 else "proj"
                      ps = psum.tile([128, 512], F32, tag=tag, bufs=2, name="p2_ps")
                      nc.tensor.matmul(
                          ps[:],
                          cu7[:, m * 128 : (m + 1) * 128],
                          wo_sb[:, 7, n * 512 : (n + 1) * 512],
                          start=True,
                          stop=True,
                      )
                      sl = slice(n * 512, (n + 1) * 512)
                      if n % 2 == 0:
                          nc.vector.tensor_scalar_mul(
                              out=osb[:, sl], in0=ps[:], scalar1=mcols[:, m : m + 1]
                          )
                      else:
                          nc.scalar.activation(
                              out=osb[:, sl],
                              in_=ps[:],
                              func=mybir.ActivationFunctionType.Copy,
                              scale=mcols[:, m : m + 1],
                          )
                  nc.sync.dma_start(
                      out=out_ds[2][m * 128 : (m + 1) * 128, :], in_=osb[:]
                  )

    nc.finalize()
    return nc


def _get_nc():
    if "nc" not in _NC_CACHE:
        _NC_CACHE["nc"] = build_nc()
    return _NC_CACHE["nc"]


def _rope_tables():
    inv_freq = 1.0 / (ROPE_THETA ** (np.arange(0, DH, 2, dtype=np.float32) / DH))
    freqs = np.arange(S, dtype=np.float32)[:, None] * inv_freq[None, :]
    emb = np.concatenate([freqs, freqs], axis=-1)  # [S, 128]
    cosT = np.ascontiguousarray(np.cos(emb).T.astype(NPBF))  # [128, S]
    sinS = np.sin(emb).T.astype(np.float32).copy()
    sinS[0:64, :] *= -1.0  # sign-folded rotate_half
    return cosT, np.ascontiguousarray(sinS.astype(NPBF))


def kernel(x, wq, bq, wk, bk, wv, bv, wo, bo, _trace=False, _tmpdir=None):
    x = np.asarray(x, dtype=np.float32)
    wq = np.asarray(wq, dtype=np.float32)
    wk = np.asarray(wk, dtype=np.float32)
    wv = np.asarray(wv, dtype=np.float32)
    wo = np.asarray(wo, dtype=np.float32)
    bq = np.asarray(bq, dtype=np.float32)
    bk = np.asarray(bk, dtype=np.float32)
    bv = np.asarray(bv, dtype=np.float32)
    bo = np.asarray(bo, dtype=np.float32)

    nc = _get_nc()
    cosT, sinS = _rope_tables()

    def qk_pack(w, g):
        ws = w[g * 1024 : (g + 1) * 1024, :]
        return np.ascontiguousarray(
            ws.reshape(HL, 128, KO, 128).transpose(0, 2, 3, 1).astype(NPBF)
        )

    packs = []
    for g in range(2):
        wv_s = wv[g * 1024 : (g + 1) * 1024, :]
        wv_p = np.ascontiguousarray(
            wv_s.reshape(HL * DH, KO, 128).transpose(1, 2, 0).astype(NPBF)
        )
        wo_s = wo[:, g * 1024 : (g + 1) * 1024]
        wo_p = np.ascontiguousarray(
            wo_s.reshape(D, HL, 128).transpose(1, 2, 0).astype(NPBF)
        )
        bq_p = np.ascontiguousarray(
            (bq[g * 1024 : (g + 1) * 1024] * QSCALE).reshape(HL, 128).T
        )
        bk_p = np.ascontiguousarray(bk[g * 1024 : (g + 1) * 1024].reshape(HL, 128).T)
        packs.append(
            dict(
                wq=qk_pack(wq, g),
                wk=qk_pack(wk, g),
                wv=wv_p,
                wo=wo_p,
                bq=bq_p,
                bk=bk_p,
            )
        )

    in_maps = []
    xts = [
        np.ascontiguousarray(x[b].T.astype(NPBF)).reshape(KO, 128, S)
        for b in range(B)
    ]
    for c in range(8):
        b, g = c // 2, c % 2
        m = dict(packs[g])
        m["xt"] = xts[b]
        m["cosT"] = cosT
        m["sinS"] = sinS
        in_maps.append(m)

    res = run_bass_kernel_spmd(
        nc,
        in_maps,
        core_ids=list(range(8)),
        trace=_trace,
        tmpdir=_tmpdir,
    )

    bo_eff = bo + wo @ bv
    out = np.empty((B, S, D), dtype=np.float32)
    for b in range(B):
        acc = np.zeros((S, D), dtype=np.float32)
        for c in (2 * b, 2 * b + 1):
            for p in range(len(P3_PARTS)):
                acc += res.results[c][f"out{p}"].astype(np.float32)
        out[b] = acc + bo_eff[None, :]
    if _trace:
        kernel.last_result = res
    return out
